# revision 1
# baseline (speedup 1.0000x reference)
"""Self-contained TRN2 kernel for nn_FLASH_ShareA_FFConvM_FlashAttn.

kernel(**inputs) takes the full (unsharded) inputs from setup_inputs() and
returns the full (B, N, D) float32 output. Internally: data-parallel over the
batch — one batch sample per NeuronCore, 8 cores, no collectives.
"""
import sys

if "/opt/trn_rl_repo" not in sys.path:
    sys.path.insert(0, "/opt/trn_rl_repo")

import numpy as np
import ml_dtypes
import concourse.bass as bass
import concourse.bacc as bacc
import concourse.mybir as mybir
import concourse.tile as tile
from concourse import bass_utils

F32 = mybir.dt.float32
BF16 = mybir.dt.bfloat16
FP8 = mybir.dt.float8e4
AF = mybir.ActivationFunctionType
OP = mybir.AluOpType

N, D, H, QK, G = 4096, 512, 2048, 128, 256
NG = N // G
NT = N // 128
KTAPS = 17
PAD = 8
E2 = 2 * D
EPS = 1e-5

# channel tiles of the depthwise convs that run on the PE (diagonal matmuls)
# instead of the vector engine; tune for engine balance.
CONV_PE_HID = frozenset({0, 1, 2, 3, 4, 5, 6, 8, 9, 10, 12, 13})
CONV_PE_O = frozenset({0, 2})
P3_ORDER = list(range(16))


def _conv_dve(nc, scratch, hpad, hpad1, dw_sb, dwi, acc):
    """acc = h + conv(h) via tensor_scalar products (4x) + tensor_tensor adds (2x)."""
    for k in range(KTAPS):
        s = k - PAD
        if s % 2 == 0:
            src, off = hpad, PAD + s
        else:
            src, off = hpad1, PAD - 1 + s
        if k == 0:
            # acc = h + w0*h_s0
            nc.vector.scalar_tensor_tensor(
                out=acc[:, :], in0=src[:, off:off + N], scalar=dw_sb[:, dwi, 0:1],
                in1=hpad[:, PAD:PAD + N], op0=OP.mult, op1=OP.add)
        else:
            nc.vector.tensor_scalar(out=scratch[:, :], in0=src[:, off:off + N],
                                    scalar1=dw_sb[:, dwi, k:k + 1], scalar2=None,
                                    op0=OP.mult)
            nc.vector.tensor_add(acc[:, :], acc[:, :], scratch[:, :])


def _conv_pe(nc, psum_pool, sbuf_pool, hpad, diag_mask, dw_sb, dwi, acc, evac):
    diag = sbuf_pool.tile([128, KTAPS, 128], BF16, tag="convdiag")
    for k in range(KTAPS):
        nc.vector.tensor_scalar(
            out=diag[:, k, :], in0=diag_mask[:, :], scalar1=dw_sb[:, dwi, k:k + 1],
            scalar2=None, op0=OP.mult)
    for cp2 in range(4):
        c0, c1 = 2 * cp2, 2 * cp2 + 1
        p0 = psum_pool.tile([128, 512], F32, tag="convpe")
        p1 = psum_pool.tile([128, 512], F32, tag="convpe")
        b0 = PAD + c0 * 512
        b1 = PAD + c1 * 512
        nc.tensor.matmul(p0[:, :], diag_mask[:, :], hpad[:, b0:b0 + 512], start=True, stop=False)
        nc.tensor.matmul(p1[:, :], diag_mask[:, :], hpad[:, b1:b1 + 512], start=True, stop=False)
        for k in range(KTAPS):
            s = k - PAD
            last = k == KTAPS - 1
            nc.tensor.matmul(p0[:, :], diag[:, k, :], hpad[:, b0 + s:b0 + s + 512],
                             start=False, stop=last)
            nc.tensor.matmul(p1[:, :], diag[:, k, :], hpad[:, b1 + s:b1 + s + 512],
                             start=False, stop=last)
        evac.activation(acc[:, c0 * 512:(c0 + 1) * 512], p0[:, :], AF.Copy)
        evac.activation(acc[:, c1 * 512:(c1 + 1) * 512], p1[:, :], AF.Copy)


def _emit(nc, tc, x, wh, wqk, wo, bh, bqk, bo, dwh, dwqk, dwo, gb, out, spill, zspill):
    from contextlib import ExitStack
    es = ExitStack()
    consts = es.enter_context(tc.tile_pool(name="consts", bufs=1))
    wh_sb = consts.tile([128, 4, H], BF16)
    nc.sync.dma_start(wh_sb[:, :, :], wh.ap())
    wqk_sb = consts.tile([128, 4, QK], BF16)
    nc.sync.dma_start(wqk_sb[:, :, :], wqk.ap())
    wo_sb = consts.tile([128, 8, D], BF16)
    nc.sync.dma_start(wo_sb[:, :, :], wo.ap())
    bh_sb = consts.tile([128, 16], F32)
    nc.sync.dma_start(bh_sb[:, :], bh.ap())
    bqk_sb = consts.tile([128, 1], F32)
    nc.sync.dma_start(bqk_sb[:, :], bqk.ap())
    bo_sb = consts.tile([128, 4], F32)
    nc.sync.dma_start(bo_sb[:, :], bo.ap())
    dwh_sb = consts.tile([128, 16, KTAPS], F32)
    nc.sync.dma_start(dwh_sb[:, :, :], dwh.ap())
    dwqk_sb = consts.tile([128, 1, KTAPS], F32)
    nc.sync.dma_start(dwqk_sb[:, :, :], dwqk.ap())
    dwo_sb = consts.tile([128, 4, KTAPS], F32)
    nc.sync.dma_start(dwo_sb[:, :, :], dwo.ap())
    gb_sb = consts.tile([128, 8], F32)
    nc.sync.dma_start(gb_sb[:, :], gb.ap())
    eps_sb = consts.tile([128, 1], F32)
    nc.vector.memset(eps_sb[:, :], EPS)

    diag_mask = None
    if CONV_PE_HID or CONV_PE_O:
        iota_row = consts.tile([128, 128], F32)
        nc.gpsimd.iota(iota_row[:, :], pattern=[[1, 128]], base=0, channel_multiplier=0,
                       allow_small_or_imprecise_dtypes=True)
        iota_p = consts.tile([128, 1], F32)
        nc.gpsimd.iota(iota_p[:, :], pattern=[[0, 1]], base=0, channel_multiplier=1,
                       allow_small_or_imprecise_dtypes=True)
        diag_mask = consts.tile([128, 128], BF16)
        nc.vector.tensor_scalar(out=diag_mask[:, :], in0=iota_row[:, :],
                                scalar1=iota_p[:, :], scalar2=None, op0=OP.is_equal)

    bigp = es.enter_context(tc.tile_pool(name="bigp", bufs=1))
    qs = es.enter_context(tc.tile_pool(name="qside", bufs=1))
    zT = bigp.tile([128, 4, N], BF16, tag="big4")
    attnT = qs.tile([128, NG, 2, G], BF16)
    lq_sb = qs.tile([128, N], BF16)
    lk_str = qs.tile([128, NT, 128], BF16)
    linkv_sb = qs.tile([128, E2], BF16)
    linku_sb = qs.tile([128, E2], BF16)

    # P0: token-shifted LayerNorm
    with tc.tile_pool(name="p0", bufs=3) as p0, \
         tc.tile_pool(name="p0s", bufs=4) as p0s:
        for tt in range(NT):
            xt = p0.tile([128, D], F32, tag="xt")
            t0 = tt * 128
            if tt == 0:
                nc.vector.memset(xt[0:1, 0:D // 2], 0.0)
                nc.sync.dma_start(xt[1:128, 0:D // 2], x[0:127, 0:D // 2])
            else:
                nc.sync.dma_start(xt[:, 0:D // 2], x[t0 - 1:t0 + 127, 0:D // 2])
            nc.sync.dma_start(xt[:, D // 2:D], x[t0:t0 + 128, D // 2:D])
            st6 = p0s.tile([128, 6], F32, tag="st6")
            nc.vector.bn_stats(st6[:, :], xt[:, :])
            mv = p0s.tile([128, 2], F32, tag="mv")
            nc.vector.bn_aggr(mv[:, :], st6[:, :])
            rstd = p0s.tile([128, 1], F32, tag="rstd")
            nc.scalar.activation(rstd[:, :], mv[:, 1:2], AF.Sqrt, bias=eps_sb[:, :], scale=1.0)
            nc.vector.reciprocal(rstd[:, :], rstd[:, :])
            nmu = p0s.tile([128, 1], F32, tag="nmu")
            nc.vector.tensor_scalar(out=nmu[:, :], in0=mv[:, 0:1], scalar1=rstd[:, :],
                                    scalar2=-1.0, op0=OP.mult, op1=OP.mult)
            zt = p0.tile([128, D], BF16, tag="zt")
            nc.vector.tensor_scalar(out=zt[:, :], in0=xt[:, :], scalar1=rstd[:, :],
                                    scalar2=nmu[:, :], op0=OP.mult, op1=OP.add)
            nc.sync.dma_start_transpose(zT[:, :, t0:t0 + 128], zt[:, :])

    # P1/P2: qk path + attention weights
    with tc.tile_pool(name="p1", bufs=1) as p1, \
         tc.tile_pool(name="p1p", bufs=2, space="PSUM") as p1p:
        qkp = p1.tile([128, 2 * PAD + N], BF16, tag="qkpad")
        nc.vector.memset(qkp[:, 0:PAD], 0.0)
        nc.vector.memset(qkp[:, PAD + N:], 0.0)
        for ch in range(8):
            ps = p1p.tile([128, 512], F32, tag="qkps")
            for kt in range(4):
                nc.tensor.matmul(ps[:, :], wqk_sb[:, kt, :], zT[:, kt, ch * 512:(ch + 1) * 512],
                                 start=(kt == 0), stop=(kt == 3))
            nc.scalar.activation(qkp[:, PAD + ch * 512:PAD + (ch + 1) * 512], ps[:, :],
                                 AF.Silu, bias=bqk_sb[:, :], scale=1.0)
        qkp1 = p1.tile([128, 2 * PAD + N], BF16, tag="qkpad1")
        nc.gpsimd.tensor_copy(qkp1[:, 0:2 * PAD + N - 2], qkp[:, 1:2 * PAD + N - 1])
        qkc = p1.tile([128, N], BF16, tag="qkc")
        qscr = p1.tile([128, N], BF16, tag="qscr")
        _conv_dve(nc, qscr, qkp, qkp1, dwqk_sb, 0, qkc)
        qq = p1.tile([128, N], BF16, tag="qq")
        qkk = p1.tile([128, N], BF16, tag="qkk")
        lkk = p1.tile([128, N], BF16, tag="lkk")
        for i, dst in ((0, qq), (1, lq_sb), (2, qkk), (3, lkk)):
            nc.vector.tensor_scalar(out=dst[:, :], in0=qkc[:, :], scalar1=gb_sb[:, i:i + 1],
                                    scalar2=gb_sb[:, 4 + i:5 + i], op0=OP.mult, op1=OP.add)
        nc.sync.dma_start_transpose(lk_str[:, :, :], lkk[:, :])

        for g in range(NG):
            for jh in range(2):
                sp = p1p.tile([128, G], F32, tag="simps")
                nc.tensor.matmul(sp[:, :], qkk[:, g * G + jh * 128: g * G + jh * 128 + 128],
                                 qq[:, g * G:(g + 1) * G], start=True, stop=True)
                rel = p1.tile([128, G], BF16, tag="rel")
                nc.scalar.activation(rel[:, :], sp[:, :], AF.Relu)
                nc.scalar.activation(attnT[:, g, jh, :], rel[:, :], AF.Square)

    # P3: hidden + conv + spill + lin_kv/lin_ku
    # strips of 4 consecutive hc tiles are batched into one [128, NT, 4, 128]
    # buffer so the spill DMA uses 1KB lines and the lin matmuls get N=512.
    spill_v = spill.ap().rearrange("(tt p) (q c4) -> p tt q c4", p=128, c4=512)
    with tc.tile_pool(name="p3", bufs=2) as p3, \
         tc.tile_pool(name="p3q", bufs=1) as p3q, \
         tc.tile_pool(name="p3p", bufs=2, space="PSUM") as p3p, \
         tc.tile_pool(name="p3lin", bufs=1, space="PSUM") as p3lin:
        kvps = p3lin.tile([128, E2], F32, tag="kvps")
        kups = p3lin.tile([128, E2], F32, tag="kups")
        state = {"strips4": None}

        def produce(hc):
            hpad = p3.tile([128, 2 * PAD + N], BF16, tag="hpad")
            nc.vector.memset(hpad[:, 0:PAD], 0.0)
            nc.vector.memset(hpad[:, PAD + N:], 0.0)
            for cp2 in range(4):
                c0 = 2 * cp2
                ps0 = p3p.tile([128, 512], F32, tag="hps")
                ps1 = p3p.tile([128, 512], F32, tag="hps")
                for kt in range(4):
                    nc.tensor.matmul(ps0[:, :], wh_sb[:, kt, hc * 128:(hc + 1) * 128],
                                     zT[:, kt, c0 * 512:(c0 + 1) * 512],
                                     start=(kt == 0), stop=(kt == 3))
                    nc.tensor.matmul(ps1[:, :], wh_sb[:, kt, hc * 128:(hc + 1) * 128],
                                     zT[:, kt, (c0 + 1) * 512:(c0 + 2) * 512],
                                     start=(kt == 0), stop=(kt == 3))
                nc.scalar.activation(hpad[:, PAD + c0 * 512:PAD + (c0 + 1) * 512], ps0[:, :],
                                     AF.Silu, bias=bh_sb[:, hc:hc + 1], scale=1.0)
                nc.scalar.activation(hpad[:, PAD + (c0 + 1) * 512:PAD + (c0 + 2) * 512],
                                     ps1[:, :], AF.Silu, bias=bh_sb[:, hc:hc + 1], scale=1.0)
            return hpad

        def convpost(hc, hpad):
            if hc % 4 == 0:
                s4_new = p3q.tile([128, NT, 4, 128], BF16, tag="strips4")
                state["strips4"] = s4_new
            strips4 = state["strips4"]
            acc = p3.tile([128, N], BF16, tag="acc")
            if hc in CONV_PE_HID:
                _conv_pe(nc, p3p, p3, hpad, diag_mask, dwh_sb, hc, acc, nc.scalar)
            else:
                hpad1 = p3q.tile([128, 2 * PAD + N], BF16, tag="hpad1")
                nc.gpsimd.tensor_copy(hpad1[:, 0:2 * PAD + N - 2], hpad[:, 1:2 * PAD + N - 1])
                scr = p3q.tile([128, N], BF16, tag="convscr")
                _conv_dve(nc, scr, hpad, hpad1, dwh_sb, hc, acc)
            nc.sync.dma_start_transpose(strips4[:, :, hc % 4, :], acc[:, :])
            if hc % 4 == 3:
                q = hc // 4
                nc.sync.dma_start(spill_v[:, :, q, :], strips4[:, :, :, :])
                dst, col = (kvps, (q % 2) * 512) if hc < 8 else (kups, (q % 2) * 512)
                for tt in range(NT):
                    nc.tensor.matmul(
                        dst[:, col:col + 512], lk_str[:, tt, :],
                        strips4[:, tt, :, :].rearrange("p a c -> p (a c)"),
                        start=(tt == 0), stop=(tt == NT - 1))

        prev = None
        for hc in P3_ORDER:
            hp = produce(hc)
            if prev is not None:
                convpost(*prev)
            prev = (hc, hp)
        convpost(*prev)
        nc.scalar.activation(linkv_sb[:, :], kvps[:, :], AF.Copy)
        nc.scalar.activation(linku_sb[:, :], kups[:, :], AF.Copy)

    # P4: attention + gating + LN_o
    zsp_v = zspill.ap().rearrange("a p t -> p a t")
    with tc.tile_pool(name="p4", bufs=2) as p4, \
         tc.tile_pool(name="p4s", bufs=4) as p4s, \
         tc.tile_pool(name="p4p", bufs=2, space="PSUM") as p4p:
        for g in range(NG):
            vg, ug = [], []
            for jh in range(2):
                vt = p4.tile([128, E2], BF16, tag=f"vg{jh}")
                nc.sync.dma_start(vt[:, :], spill[g * G + jh * 128: g * G + jh * 128 + 128, 0:E2])
                ut = p4.tile([128, E2], BF16, tag=f"ug{jh}")
                nc.sync.dma_start(ut[:, :], spill[g * G + jh * 128: g * G + jh * 128 + 128, E2:H])
                vg.append(vt)
                ug.append(ut)
            for it in range(2):
                ap_ = p4p.tile([128, 2 * E2], F32, tag="attps")
                islice = slice(g * G + it * 128, g * G + it * 128 + 128)
                for half, (grp, lin) in enumerate(((vg, linkv_sb), (ug, linku_sb))):
                    base = half * E2
                    for e in range(2):
                        for jh in range(2):
                            nc.tensor.matmul(ap_[:, base + e * 512:base + (e + 1) * 512],
                                             attnT[:, g, jh, it * 128:it * 128 + 128],
                                             grp[jh][:, e * 512:(e + 1) * 512],
                                             start=(jh == 0), stop=False)
                        nc.tensor.matmul(ap_[:, base + e * 512:base + (e + 1) * 512],
                                         lq_sb[:, islice], lin[:, e * 512:(e + 1) * 512],
                                         start=False, stop=True)
                avs = p4.tile([128, E2], BF16, tag="avs")
                nc.scalar.activation(avs[:, :], ap_[:, 0:E2], AF.Copy)
                aus = p4.tile([128, E2], BF16, tag="aus")
                nc.scalar.activation(aus[:, :], ap_[:, E2:2 * E2], AF.Copy)
                t1 = p4.tile([128, E2], BF16, tag="t1")
                nc.vector.tensor_mul(t1[:, :], ug[it][:, :], avs[:, :])
                sg = p4.tile([128, E2], BF16, tag="sg")
                nc.scalar.activation(sg[:, :], t1[:, :], AF.Sigmoid)
                t2 = p4.tile([128, E2], BF16, tag="t2")
                nc.vector.tensor_mul(t2[:, :], vg[it][:, :], aus[:, :])
                go = p4.tile([128, E2], BF16, tag="go")
                sumg = p4s.tile([128, 1], F32, tag="sumg")
                nc.vector.scalar_tensor_tensor(out=go[:, :], in0=t2[:, :], scalar=1.0,
                                               in1=sg[:, :], op0=OP.mult, op1=OP.mult,
                                               accum_out=sumg[:, :])
                g2 = p4.tile([128, E2], BF16, tag="g2")
                sumg2 = p4s.tile([128, 1], F32, tag="sumg2")
                nc.scalar.activation(g2[:, :], go[:, :], AF.Square, accum_out=sumg2[:, :])
                mean = p4s.tile([128, 1], F32, tag="mean")
                nc.vector.tensor_scalar_mul(mean[:, :], sumg[:, :], 1.0 / E2)
                mm = p4s.tile([128, 1], F32, tag="mm")
                nc.vector.tensor_scalar(out=mm[:, :], in0=mean[:, :], scalar1=mean[:, :],
                                        scalar2=-1.0, op0=OP.mult, op1=OP.mult)
                var = p4s.tile([128, 1], F32, tag="var")
                nc.vector.tensor_scalar(out=var[:, :], in0=sumg2[:, :], scalar1=1.0 / E2,
                                        scalar2=mm[:, :], op0=OP.mult, op1=OP.add)
                rstd = p4s.tile([128, 1], F32, tag="rstd4")
                nc.scalar.activation(rstd[:, :], var[:, :], AF.Sqrt, bias=eps_sb[:, :], scale=1.0)
                nc.vector.reciprocal(rstd[:, :], rstd[:, :])
                nmu = p4s.tile([128, 1], F32, tag="nmu4")
                nc.vector.tensor_scalar(out=nmu[:, :], in0=mean[:, :], scalar1=rstd[:, :],
                                        scalar2=-1.0, op0=OP.mult, op1=OP.mult)
                zo = p4.tile([128, E2], BF16, tag="zo")
                nc.vector.tensor_scalar(out=zo[:, :], in0=go[:, :], scalar1=rstd[:, :],
                                        scalar2=nmu[:, :], op0=OP.mult, op1=OP.add)
                tti = g * 2 + it
                zot_t = p4.tile([128, 8, 128], BF16, tag="zot")
                nc.sync.dma_start_transpose(zot_t[:, :, :], zo[:, :])
                nc.sync.dma_start(zsp_v[:, :, tti * 128:(tti + 1) * 128], zot_t[:, :, :])

    # P5: output FFConvM
    vo_big = bigp.tile([128, NT, 4, 128], BF16, tag="big4")
    with tc.tile_pool(name="p5", bufs=2) as p5, \
         tc.tile_pool(name="p5q", bufs=1) as p5q, \
         tc.tile_pool(name="p5p", bufs=3, space="PSUM") as p5p:

        def produce5(oc):
            hpad = p5.tile([128, 2 * PAD + N], BF16, tag="hpad5")
            nc.vector.memset(hpad[:, 0:PAD], 0.0)
            nc.vector.memset(hpad[:, PAD + N:], 0.0)
            for cp2 in range(4):
                c0 = 2 * cp2
                zoc = p5.tile([128, 8, 1024], BF16, tag="zoc")
                nc.sync.dma_start(zoc[:, :, :], zsp_v[:, :, c0 * 512:(c0 + 2) * 512])
                ps0 = p5p.tile([128, 512], F32, tag="ops")
                ps1 = p5p.tile([128, 512], F32, tag="ops")
                for kt in range(8):
                    nc.tensor.matmul(ps0[:, :], wo_sb[:, kt, oc * 128:(oc + 1) * 128],
                                     zoc[:, kt, 0:512], start=(kt == 0), stop=(kt == 7))
                    nc.tensor.matmul(ps1[:, :], wo_sb[:, kt, oc * 128:(oc + 1) * 128],
                                     zoc[:, kt, 512:1024], start=(kt == 0), stop=(kt == 7))
                nc.scalar.activation(hpad[:, PAD + c0 * 512:PAD + (c0 + 1) * 512], ps0[:, :],
                                     AF.Silu, bias=bo_sb[:, oc:oc + 1], scale=1.0)
                nc.scalar.activation(hpad[:, PAD + (c0 + 1) * 512:PAD + (c0 + 2) * 512],
                                     ps1[:, :], AF.Silu, bias=bo_sb[:, oc:oc + 1], scale=1.0)
            return hpad

        def convpost5(oc, hpad):
            acc = p5.tile([128, N], BF16, tag="acc5")
            if oc in CONV_PE_O:
                _conv_pe(nc, p5p, p5, hpad, diag_mask, dwo_sb, oc, acc, nc.scalar)
            else:
                hpad1 = p5q.tile([128, 2 * PAD + N], BF16, tag="hpad51")
                nc.gpsimd.tensor_copy(hpad1[:, 0:2 * PAD + N - 2], hpad[:, 1:2 * PAD + N - 1])
                scr = p5q.tile([128, N], BF16, tag="convscr5")
                _conv_dve(nc, scr, hpad, hpad1, dwo_sb, oc, acc)
            nc.sync.dma_start_transpose(vo_big[:, :, oc, :], acc[:, :])

        prev = None
        for oc in range(4):
            hp = produce5(oc)
            if prev is not None:
                convpost5(*prev)
            prev = (oc, hp)
        convpost5(*prev)

    # P6: residual
    with tc.tile_pool(name="p6", bufs=3) as p6:
        for tt in range(NT):
            xt = p6.tile([128, D], F32, tag="xt6")
            nc.sync.dma_start(xt[:, :], x[tt * 128:(tt + 1) * 128, :])
            of = p6.tile([128, D], F32, tag="of")
            nc.gpsimd.tensor_add(of[:, :], xt[:, :],
                                 vo_big[:, tt, :, :].rearrange("p a c -> p (a c)"))
            nc.sync.dma_start(out[tt * 128:(tt + 1) * 128, :], of[:, :])
    es.close()


def _build_nc():
    nc = bacc.Bacc("TRN2", target_bir_lowering=False, debug=False)
    x = nc.dram_tensor("x", [N, D], F32, kind="ExternalInput")
    wh = nc.dram_tensor("wh", [128, 4, H], BF16, kind="ExternalInput")
    wqk = nc.dram_tensor("wqk", [128, 4, QK], BF16, kind="ExternalInput")
    wo = nc.dram_tensor("wo", [128, 8, D], BF16, kind="ExternalInput")
    bh = nc.dram_tensor("bh", [128, 16], F32, kind="ExternalInput")
    bqk = nc.dram_tensor("bqk", [128, 1], F32, kind="ExternalInput")
    bo = nc.dram_tensor("bo", [128, 4], F32, kind="ExternalInput")
    dwh = nc.dram_tensor("dwh", [128, 16, KTAPS], F32, kind="ExternalInput")
    dwqk = nc.dram_tensor("dwqk", [128, 1, KTAPS], F32, kind="ExternalInput")
    dwo = nc.dram_tensor("dwo", [128, 4, KTAPS], F32, kind="ExternalInput")
    gb = nc.dram_tensor("gb", [128, 8], F32, kind="ExternalInput")
    out = nc.dram_tensor("out", [N, D], F32, kind="ExternalOutput")
    spill = nc.dram_tensor("spill", [N, H], BF16)
    zspill = nc.dram_tensor("zspill", [8, 128, N], BF16)
    with tile.TileContext(nc) as tc:
        _emit(nc, tc, x, wh, wqk, wo, bh, bqk, bo, dwh, dwqk, dwo, gb, out, spill, zspill)
    nc.compile()
    return nc


def prep_inputs(inputs):
    f32 = np.float32
    bf = ml_dtypes.bfloat16
    W_h = np.asarray(inputs["W_h"], f32)
    W_qk = np.asarray(inputs["W_qk"], f32)
    W_o = np.asarray(inputs["W_o"], f32)
    whp = np.asarray(inputs["ln_h_g"], f32)[:, None] * W_h
    bhp = np.asarray(inputs["ln_h_b"], f32) @ W_h + np.asarray(inputs["b_h"], f32)
    wqkp = np.asarray(inputs["ln_qk_g"], f32)[:, None] * W_qk
    bqkp = np.asarray(inputs["ln_qk_b"], f32) @ W_qk + np.asarray(inputs["b_qk"], f32)
    wop = np.asarray(inputs["ln_o_g"], f32)[:, None] * W_o
    bop = np.asarray(inputs["ln_o_b"], f32) @ W_o + np.asarray(inputs["b_o"], f32)
    gamma = np.asarray(inputs["gamma"], f32).copy()
    beta = np.asarray(inputs["beta"], f32).copy()
    gamma[0] /= G
    beta[0] /= G
    gamma[3] /= N
    beta[3] /= N

    def lhsT(w, ktiles):
        return np.ascontiguousarray(w.reshape(ktiles, 128, -1).transpose(1, 0, 2)).astype(bf)

    def chan(v, ntiles):
        return np.ascontiguousarray(v.reshape(ntiles, 128).T).astype(f32)

    def dwl(dw, ntiles):
        return np.ascontiguousarray(
            dw.T.reshape(ntiles, 128, KTAPS).transpose(1, 0, 2)).astype(f32)

    return {
        "wh": lhsT(whp, 4), "wqk": lhsT(wqkp, 4), "wo": lhsT(wop, 8),
        "bh": chan(bhp, 16), "bqk": chan(bqkp, 1), "bo": chan(bop, 4),
        "dwh": dwl(np.asarray(inputs["dw_h"], f32), 16),
        "dwqk": dwl(np.asarray(inputs["dw_qk"], f32), 1),
        "dwo": dwl(np.asarray(inputs["dw_o"], f32), 4),
        "gb": np.concatenate([gamma.T, beta.T], axis=1).astype(f32),
    }


_NC = None


def get_nc():
    global _NC
    if _NC is None:
        _NC = _build_nc()
    return _NC


def make_in_maps(inputs):
    x = np.asarray(inputs["x"], np.float32)
    B = x.shape[0]
    prep = prep_inputs(inputs)
    return [{"x": np.ascontiguousarray(x[b]), **prep} for b in range(B)]


def kernel(**inputs):
    nc = get_nc()
    in_maps = make_in_maps(inputs)
    res = bass_utils.run_bass_kernel_spmd(nc, in_maps, core_ids=list(range(8)))
    out = np.stack([res.results[b]["out"] for b in range(8)], axis=0)
    return out.astype(np.float32)



# revision 7
# speedup vs baseline: 1.3091x; 1.3091x over previous
"""Self-contained TRN2 kernel for nn_FLASH_ShareA_FFConvM_FlashAttn.

kernel(**inputs) takes the full (unsharded) inputs from setup_inputs() and
returns the full (B, N, D) float32 output. Internally: data-parallel over the
batch — one batch sample per NeuronCore, 8 cores, no collectives.

v2: all heavy matmuls in fp8 DoubleRow (paired k-tiles / paired conv taps),
depthwise convs fully on the PE as paired diagonal matmuls, attention weights
pre-scaled by 2^30 to stay in fp8 range, deferred output LayerNorm, and the
zspill round-trip replaced by an SBUF-resident transposed buffer.
"""
import sys

if "/opt/trn_rl_repo" not in sys.path:
    sys.path.insert(0, "/opt/trn_rl_repo")

import numpy as np
import ml_dtypes
import concourse.bass as bass
import concourse.bacc as bacc
import concourse.mybir as mybir
import concourse.tile as tile
from concourse import bass_utils
from concourse.ap import AP

F32 = mybir.dt.float32
BF16 = mybir.dt.bfloat16
FP8 = mybir.dt.float8e4
AF = mybir.ActivationFunctionType
OP = mybir.AluOpType
DR = mybir.MatmulPerfMode.DoubleRow

N, D, H, QK, G = 4096, 512, 2048, 128, 256
NG = N // G
NT = N // 128
KTAPS = 17
PAD = 8
NPADBUF = N + 2 * PAD  # fp8/bf16 padded conv input length (max tap read = N+15)
E2 = 2 * D
EPS = 1e-5
NCH = 21  # conv channel tiles: hid 0..15, out 16..19, qk 20
# conv tap pairs with stride-4 spacing (DR rows must be >=4 fp8 elements apart)
PAIRS = [(0, 4), (1, 5), (2, 6), (3, 7), (8, 12), (9, 13), (10, 14), (11, 15)]
ASCALE = float(2 ** 30)      # attention-weight scale kept inside psum
RELUSC = float(2 ** 15)      # sqrt(ASCALE), applied before squaring
GRP = 16                     # P4 deferred-LN batch size (iterations)


def _pair_ap(t, off, n):
    """Overlapping [128, 2, n] moving AP: row j reads t[:, off+4j : off+4j+n]."""
    base = t[:, 0:1]
    return AP(base.tensor, base.offset + off, [list(base.ap[0]), [4, 2], [1, n]])


def _emit_conv(nc, pool, dg, h8t, hb, acc):
    """acc[:, c] = h + conv(h): 8 DR tap pairs + tap16 on PE, identity on DVE."""
    for c in range(8):
        cb = c * 512
        cp = pool.tile([128, 512], F32, tag="convps")
        for pr in range(8):
            nc.tensor.matmul(cp[:, :], dg[:, pr, :, :], _pair_ap(h8t, PAIRS[pr][0] + cb, 512),
                             start=(pr == 0), stop=False, perf_mode=DR)
        nc.tensor.matmul(cp[:, :], dg[:, 8, 0, :], h8t[:, 16 + cb:16 + cb + 512],
                         start=False, stop=True)
        nc.vector.tensor_tensor(out=acc[:, cb:cb + 512], in0=cp[:, :],
                                in1=hb[:, PAD + cb:PAD + cb + 512], op=OP.add)


def _emit(nc, tc, x, wh8, wqk8, wo8, bh, bqk, bo, gb, diag, out, spill):
    from contextlib import ExitStack
    es = ExitStack()
    consts = es.enter_context(tc.tile_pool(name="consts", bufs=1))
    wh_sb = consts.tile([128, 2, 2, H], FP8)
    nc.sync.dma_start(wh_sb[:, :, :, :], wh8.ap())
    wqk_sb = consts.tile([128, 2, 2, QK], FP8)
    nc.sync.dma_start(wqk_sb[:, :, :, :], wqk8.ap())
    wo_sb = consts.tile([128, 4, 2, D], FP8)
    nc.sync.dma_start(wo_sb[:, :, :, :], wo8.ap())
    bh_sb = consts.tile([128, 16], F32)
    nc.sync.dma_start(bh_sb[:, :], bh.ap())
    bqk_sb = consts.tile([128, 1], F32)
    nc.sync.dma_start(bqk_sb[:, :], bqk.ap())
    bo_sb = consts.tile([128, 4], F32)
    nc.sync.dma_start(bo_sb[:, :], bo.ap())
    gb_sb = consts.tile([128, 8], F32)
    nc.sync.dma_start(gb_sb[:, :], gb.ap())
    eps_sb = consts.tile([128, 1], F32)
    nc.vector.memset(eps_sb[:, :], EPS)

    outer = es.enter_context(tc.tile_pool(name="outer", bufs=1))
    attnT8 = outer.tile([128, NG, 2, G], FP8)
    lq_sb = outer.tile([128, N], BF16)
    lk_str = outer.tile([128, NT, 128], BF16)
    linkv_sb = outer.tile([128, E2], BF16)
    linku_sb = outer.tile([128, E2], BF16)
    sums = outer.tile([128, 32], F32)
    sumsq = outer.tile([128, 32], F32)
    spill_v = spill.ap().rearrange("(tt p) (q c4) -> p tt q c4", p=128, c4=512)

    es2 = ExitStack()
    zpool = es2.enter_context(tc.tile_pool(name="zpool", bufs=1))
    zT8 = []
    for c in range(8):
        zT8c = zpool.tile([128, 4, 512], FP8, tag=f"zT8_{c}")
        zT8.append(zT8c)

    # ---------------- P0: token-shifted LayerNorm -> zT8 chunks ----------------
    with tc.tile_pool(name="p0", bufs=3) as p0, \
         tc.tile_pool(name="p0z", bufs=3) as p0z, \
         tc.tile_pool(name="p0s", bufs=4) as p0s:
        ztc = None
        for tt in range(NT):
            xt = p0.tile([128, D], F32, tag="xt")
            t0 = tt * 128
            if tt == 0:
                nc.vector.memset(xt[0:1, 0:D // 2], 0.0)
                nc.gpsimd.dma_start(xt[1:128, 0:D // 2], x[0:127, 0:D // 2])
            else:
                nc.gpsimd.dma_start(xt[:, 0:D // 2], x[t0 - 1:t0 + 127, 0:D // 2])
            nc.gpsimd.dma_start(xt[:, D // 2:D], x[t0:t0 + 128, D // 2:D])
            st6 = p0s.tile([128, 6], F32, tag="st6")
            nc.vector.bn_stats(st6[:, :], xt[:, :])
            mv = p0s.tile([128, 2], F32, tag="mv")
            nc.vector.bn_aggr(mv[:, :], st6[:, :])
            rstd = p0s.tile([128, 1], F32, tag="rstd")
            nc.scalar.activation(rstd[:, :], mv[:, 1:2], AF.Sqrt, bias=eps_sb[:, :], scale=1.0)
            nc.vector.reciprocal(rstd[:, :], rstd[:, :])
            nmu = p0s.tile([128, 1], F32, tag="nmu")
            nc.vector.tensor_scalar(out=nmu[:, :], in0=mv[:, 0:1], scalar1=rstd[:, :],
                                    scalar2=-1.0, op0=OP.mult, op1=OP.mult)
            zt = p0.tile([128, D], BF16, tag="zt")
            nc.scalar.activation(zt[:, :], xt[:, :], AF.Identity,
                                 bias=nmu[:, :], scale=rstd[:, :])
            if tt % 4 == 0:
                ztc = p0z.tile([128, 4, 512], BF16, tag="ztc")
            nc.sync.dma_start_transpose(ztc[:, :, (tt % 4) * 128:(tt % 4) * 128 + 128],
                                        zt[:, :])
            if tt % 4 == 3:
                nc.scalar.activation(zT8[tt // 4][:, :, :], ztc[:, :, :], AF.Copy)

    # ---------------- P1: qk path -> attnT8 / lq / lk_str ----------------
    with tc.tile_pool(name="p1", bufs=1) as p1, \
         tc.tile_pool(name="p1d", bufs=1) as p1d, \
         tc.tile_pool(name="p1p", bufs=2, space="PSUM") as p1p, \
         tc.tile_pool(name="p1cp", bufs=2, space="PSUM") as p1cp:
        dgq = p1d.tile([128, 9, 2, 128], FP8, tag="dgq")
        nc.sync.dma_start(dgq[:, :, :, :], diag.ap()[:, 20, :, :, :])
        qkp = p1.tile([128, NPADBUF], BF16, tag="qkpad")
        nc.vector.memset(qkp[:, 0:PAD], 0.0)
        nc.vector.memset(qkp[:, PAD + N:], 0.0)
        q8p = p1.tile([128, NPADBUF], FP8, tag="qk8pad")
        for ch in range(8):
            ps = p1p.tile([128, 512], F32, tag="qkps")
            for pr in range(2):
                nc.tensor.matmul(ps[:, :], wqk_sb[:, pr, :, :],
                                 zT8[ch][:, 2 * pr:2 * pr + 2, :],
                                 start=(pr == 0), stop=(pr == 1), perf_mode=DR)
            nc.scalar.activation(qkp[:, PAD + ch * 512:PAD + (ch + 1) * 512], ps[:, :],
                                 AF.Silu, bias=bqk_sb[:, :], scale=1.0)
        nc.scalar.activation(q8p[:, :], qkp[:, :], AF.Copy)
        qkc = p1.tile([128, N], BF16, tag="qkc")
        _emit_conv(nc, p1cp, dgq, q8p, qkp, qkc)
        qq = p1.tile([128, N], BF16, tag="qq")
        qkk = p1.tile([128, N], BF16, tag="qkk")
        lkk = p1.tile([128, N], BF16, tag="lkk")
        for i, dst in ((0, qq), (1, lq_sb), (2, qkk), (3, lkk)):
            nc.vector.tensor_scalar(out=dst[:, :], in0=qkc[:, :], scalar1=gb_sb[:, i:i + 1],
                                    scalar2=gb_sb[:, 4 + i:5 + i], op0=OP.mult, op1=OP.add)
        nc.sync.dma_start_transpose(lk_str[:, :, :], lkk[:, :])

        for g in range(NG):
            for jh in range(2):
                sp = p1p.tile([128, G], F32, tag="simps")
                nc.tensor.matmul(sp[:, :], qkk[:, g * G + jh * 128: g * G + jh * 128 + 128],
                                 qq[:, g * G:(g + 1) * G], start=True, stop=True)
                rel = p1.tile([128, G], BF16, tag="rel")
                nc.scalar.activation(rel[:, :], sp[:, :], AF.Relu, scale=RELUSC)
                nc.scalar.activation(attnT8[:, g, jh, :], rel[:, :], AF.Square)

    # ---------------- P3: hidden FFConvM -> spill + lin_kv/lin_ku ----------------
    with tc.tile_pool(name="p3", bufs=2) as p3, \
         tc.tile_pool(name="p3d", bufs=3) as p3d, \
         tc.tile_pool(name="p3q", bufs=1) as p3q, \
         tc.tile_pool(name="p3p", bufs=2, space="PSUM") as p3p, \
         tc.tile_pool(name="p3cp", bufs=2, space="PSUM") as p3cp, \
         tc.tile_pool(name="p3lin", bufs=2, space="PSUM") as p3lin:
        state = {"strips4": None}

        def produce(hc):
            dg = p3d.tile([128, 9, 2, 128], FP8, tag="dg")
            nc.sync.dma_start(dg[:, :, :, :], diag.ap()[:, hc, :, :, :])
            hb = p3.tile([128, NPADBUF], BF16, tag="hpad")
            nc.vector.memset(hb[:, 0:PAD], 0.0)
            nc.vector.memset(hb[:, PAD + N:], 0.0)
            for cp2 in range(4):
                for k in range(2):
                    c = 2 * cp2 + k
                    ps = p3p.tile([128, 512], F32, tag="hps")
                    for pr in range(2):
                        nc.tensor.matmul(ps[:, :], wh_sb[:, pr, :, hc * 128:(hc + 1) * 128],
                                         zT8[c][:, 2 * pr:2 * pr + 2, :],
                                         start=(pr == 0), stop=(pr == 1), perf_mode=DR)
                    nc.scalar.activation(hb[:, PAD + c * 512:PAD + (c + 1) * 512], ps[:, :],
                                         AF.Silu, bias=bh_sb[:, hc:hc + 1], scale=1.0)
            h8 = p3.tile([128, NPADBUF], FP8, tag="h8pad")
            nc.scalar.activation(h8[:, :], hb[:, :], AF.Copy)
            return dg, hb, h8

        def convpost(hc, dg, hb, h8):
            if hc % 4 == 0:
                s4_new = p3q.tile([128, NT, 4, 128], BF16, tag="strips4")
                state["strips4"] = s4_new
            strips4 = state["strips4"]
            acc = p3.tile([128, N], BF16, tag="acc")
            _emit_conv(nc, p3cp, dg, h8, hb, acc)
            nc.sync.dma_start_transpose(strips4[:, :, hc % 4, :], acc[:, :])
            if hc % 4 == 3:
                q = hc // 4
                nc.gpsimd.dma_start(spill_v[:, :, q, :], strips4[:, :, :, :])
                kvp = p3lin.tile([128, 512], F32, tag="kvps")
                for tt in range(NT):
                    nc.tensor.matmul(
                        kvp[:, :], lk_str[:, tt, :],
                        strips4[:, tt, :, :].rearrange("p a c -> p (a c)"),
                        start=(tt == 0), stop=(tt == NT - 1))
                dst = linkv_sb if q < 2 else linku_sb
                nc.scalar.activation(dst[:, (q % 2) * 512:(q % 2) * 512 + 512],
                                     kvp[:, :], AF.Copy)

        prev = None
        for hc in range(16):
            pr = produce(hc)
            if prev is not None:
                convpost(*prev)
            prev = (hc, *pr)
        convpost(*prev)

    es2.close()  # frees zT8 chunks before the P4/P5 pools open

    # ---------------- P4: attention apply + gating (deferred LN) ----------------
    with tc.tile_pool(name="mid", bufs=1) as mid:
        zoT8 = mid.tile([128, 8, N], FP8)
        vo_big = mid.tile([128, NT, 4, 128], BF16)
        with tc.tile_pool(name="p4", bufs=2) as p4, \
             tc.tile_pool(name="p4g", bufs=GRP + 2) as p4g, \
             tc.tile_pool(name="p4s", bufs=3) as p4s, \
             tc.tile_pool(name="p4p", bufs=2, space="PSUM") as p4p:
            govu = []   # (go, vt?, ...) per pending it in current group
            for g in range(NG):
                vt, ut = [], []
                for jh in range(2):
                    vtj = p4.tile([128, E2], BF16, tag=f"vg{jh}")
                    nc.gpsimd.dma_start(vtj[:, :], spill[g * G + jh * 128: g * G + jh * 128 + 128, 0:E2])
                    utj = p4.tile([128, E2], BF16, tag=f"ug{jh}")
                    nc.gpsimd.dma_start(utj[:, :], spill[g * G + jh * 128: g * G + jh * 128 + 128, E2:H])
                    vt.append(vtj)
                    ut.append(utj)
                vt8 = p4.tile([128, 2, E2], FP8, tag="vt8")
                ut8 = p4.tile([128, 2, E2], FP8, tag="ut8")
                for jh in range(2):
                    nc.vector.tensor_copy(vt8[:, jh, :], vt[jh][:, :])
                    nc.scalar.activation(ut8[:, jh, :], ut[jh][:, :], AF.Copy)
                for it in range(2):
                    idx = g * 2 + it
                    islice = slice(g * G + it * 128, g * G + it * 128 + 128)
                    avp = p4p.tile([128, E2], F32, tag="avps")
                    aup = p4p.tile([128, E2], F32, tag="aups")
                    for dst, m8, lin in ((avp, vt8, linkv_sb), (aup, ut8, linku_sb)):
                        for e in range(2):
                            nc.tensor.matmul(dst[:, e * 512:(e + 1) * 512],
                                             attnT8[:, g, :, it * 128:it * 128 + 128],
                                             m8[:, :, e * 512:(e + 1) * 512],
                                             start=True, stop=False, perf_mode=DR)
                            nc.tensor.matmul(dst[:, e * 512:(e + 1) * 512],
                                             lq_sb[:, islice], lin[:, e * 512:(e + 1) * 512],
                                             start=False, stop=True)
                    t1 = p4s.tile([128, E2], BF16, tag="t1")
                    nc.vector.scalar_tensor_tensor(out=t1[:, :], in0=avp[:, :],
                                                   scalar=1.0 / ASCALE, in1=ut[it][:, :],
                                                   op0=OP.mult, op1=OP.mult)
                    sg = p4s.tile([128, E2], BF16, tag="sg")
                    nc.scalar.activation(sg[:, :], t1[:, :], AF.Sigmoid)
                    t2 = p4s.tile([128, E2], BF16, tag="t2")
                    nc.vector.scalar_tensor_tensor(out=t2[:, :], in0=aup[:, :],
                                                   scalar=1.0 / ASCALE, in1=vt[it][:, :],
                                                   op0=OP.mult, op1=OP.mult)
                    go = p4g.tile([128, E2], BF16, tag="go")
                    nc.vector.scalar_tensor_tensor(out=go[:, :], in0=t2[:, :], scalar=1.0,
                                                   in1=sg[:, :], op0=OP.mult, op1=OP.mult,
                                                   accum_out=sums[:, idx:idx + 1])
                    jnk = p4s.tile([128, E2], BF16, tag="jnk")
                    nc.scalar.activation(jnk[:, :], go[:, :], AF.Square,
                                         accum_out=sumsq[:, idx:idx + 1])
                    govu.append(go)
                    if len(govu) == GRP:
                        _p4_norm(nc, tc, p4s, govu, sums, sumsq, eps_sb, zoT8,
                                 idx - GRP + 1)
                        govu = []

        # ---------------- P5: output FFConvM ----------------
        with tc.tile_pool(name="p5", bufs=2) as p5, \
             tc.tile_pool(name="p5d", bufs=2) as p5d, \
             tc.tile_pool(name="p5p", bufs=2, space="PSUM") as p5p, \
             tc.tile_pool(name="p5cp", bufs=2, space="PSUM") as p5cp:
            def produce5(oc):
                dg = p5d.tile([128, 9, 2, 128], FP8, tag="dg5")
                nc.sync.dma_start(dg[:, :, :, :], diag.ap()[:, 16 + oc, :, :, :])
                hb = p5.tile([128, NPADBUF], BF16, tag="hpad5")
                nc.vector.memset(hb[:, 0:PAD], 0.0)
                nc.vector.memset(hb[:, PAD + N:], 0.0)
                for c in range(8):
                    ps = p5p.tile([128, 512], F32, tag="ops")
                    for pr in range(4):
                        nc.tensor.matmul(ps[:, :], wo_sb[:, pr, :, oc * 128:(oc + 1) * 128],
                                         zoT8[:, 2 * pr:2 * pr + 2, c * 512:(c + 1) * 512],
                                         start=(pr == 0), stop=(pr == 3), perf_mode=DR)
                    nc.scalar.activation(hb[:, PAD + c * 512:PAD + (c + 1) * 512], ps[:, :],
                                         AF.Silu, bias=bo_sb[:, oc:oc + 1], scale=1.0)
                h8 = p5.tile([128, NPADBUF], FP8, tag="h85")
                nc.scalar.activation(h8[:, :], hb[:, :], AF.Copy)
                return dg, hb, h8

            def convpost5(oc, dg, hb, h8):
                acc = p5.tile([128, N], BF16, tag="acc5")
                _emit_conv(nc, p5cp, dg, h8, hb, acc)
                nc.sync.dma_start_transpose(vo_big[:, :, oc, :], acc[:, :])

            prev = None
            for oc in range(4):
                pr = produce5(oc)
                if prev is not None:
                    convpost5(*prev)
                prev = (oc, *pr)
            convpost5(*prev)

        # ---------------- P6: residual ----------------
        with tc.tile_pool(name="p6", bufs=3) as p6:
            for tt in range(NT):
                xt = p6.tile([128, D], F32, tag="xt6")
                nc.gpsimd.dma_start(xt[:, :], x[tt * 128:(tt + 1) * 128, :])
                of = p6.tile([128, D], F32, tag="of")
                nc.vector.tensor_tensor(out=of[:, :], in0=xt[:, :],
                                        in1=vo_big[:, tt, :, :].rearrange("p a c -> p (a c)"),
                                        op=OP.add)
                nc.gpsimd.dma_start(out[tt * 128:(tt + 1) * 128, :], of[:, :])
    es.close()


def _p4_norm(nc, tc, pool, gos, sums, sumsq, eps_sb, zoT8, idx0):
    """Deferred LayerNorm for GRP gating tiles: batched stats then per-tile
    normalize + transpose + fp8 convert."""
    n = len(gos)
    sl = slice(idx0, idx0 + n)
    mean = pool.tile([128, n], F32, tag="mean")
    nc.vector.tensor_scalar(out=mean[:, :], in0=sums[:, sl], scalar1=1.0 / E2,
                            scalar2=None, op0=OP.mult)
    msq = pool.tile([128, n], F32, tag="msq")
    nc.vector.tensor_tensor(out=msq[:, :], in0=mean[:, :], in1=mean[:, :], op=OP.mult)
    var = pool.tile([128, n], F32, tag="var")
    nc.vector.scalar_tensor_tensor(out=var[:, :], in0=sumsq[:, sl], scalar=1.0 / E2,
                                   in1=msq[:, :], op0=OP.mult, op1=OP.subtract)
    rstd = pool.tile([128, n], F32, tag="rstdn")
    nc.scalar.activation(rstd[:, :], var[:, :], AF.Sqrt, bias=eps_sb[:, :], scale=1.0)
    nc.vector.reciprocal(rstd[:, :], rstd[:, :])
    nmu = pool.tile([128, n], F32, tag="nmun")
    nc.vector.tensor_tensor(out=nmu[:, :], in0=mean[:, :], in1=rstd[:, :], op=OP.mult)
    nc.vector.tensor_scalar(out=nmu[:, :], in0=nmu[:, :], scalar1=-1.0,
                            scalar2=None, op0=OP.mult)
    for j, go in enumerate(gos):
        tti = idx0 + j
        zo = pool.tile([128, E2], BF16, tag="zon")
        nc.scalar.activation(zo[:, :], go[:, :], AF.Identity,
                             bias=nmu[:, j:j + 1], scale=rstd[:, j:j + 1])
        zot = pool.tile([128, 8, 128], BF16, tag="zot")
        nc.sync.dma_start_transpose(zot[:, :, :], zo[:, :])
        nc.scalar.activation(zoT8[:, :, tti * 128:(tti + 1) * 128], zot[:, :, :], AF.Copy)


def _build_nc():
    nc = bacc.Bacc("TRN2", target_bir_lowering=False, debug=False)
    x = nc.dram_tensor("x", [N, D], F32, kind="ExternalInput")
    wh8 = nc.dram_tensor("wh8", [128, 2, 2, H], FP8, kind="ExternalInput")
    wqk8 = nc.dram_tensor("wqk8", [128, 2, 2, QK], FP8, kind="ExternalInput")
    wo8 = nc.dram_tensor("wo8", [128, 4, 2, D], FP8, kind="ExternalInput")
    bh = nc.dram_tensor("bh", [128, 16], F32, kind="ExternalInput")
    bqk = nc.dram_tensor("bqk", [128, 1], F32, kind="ExternalInput")
    bo = nc.dram_tensor("bo", [128, 4], F32, kind="ExternalInput")
    gb = nc.dram_tensor("gb", [128, 8], F32, kind="ExternalInput")
    diag = nc.dram_tensor("diag", [128, NCH, 9, 2, 128], FP8, kind="ExternalInput")
    out = nc.dram_tensor("out", [N, D], F32, kind="ExternalOutput")
    spill = nc.dram_tensor("spill", [N, H], BF16)
    with tile.TileContext(nc) as tc:
        _emit(nc, tc, x, wh8, wqk8, wo8, bh, bqk, bo, gb, diag, out, spill)
    nc.compile()
    return nc


def prep_inputs(inputs):
    f32 = np.float32
    fp8 = ml_dtypes.float8_e4m3
    W_h = np.asarray(inputs["W_h"], f32)
    W_qk = np.asarray(inputs["W_qk"], f32)
    W_o = np.asarray(inputs["W_o"], f32)
    whp = np.asarray(inputs["ln_h_g"], f32)[:, None] * W_h
    bhp = np.asarray(inputs["ln_h_b"], f32) @ W_h + np.asarray(inputs["b_h"], f32)
    wqkp = np.asarray(inputs["ln_qk_g"], f32)[:, None] * W_qk
    bqkp = np.asarray(inputs["ln_qk_b"], f32) @ W_qk + np.asarray(inputs["b_qk"], f32)
    wop = np.asarray(inputs["ln_o_g"], f32)[:, None] * W_o
    bop = np.asarray(inputs["ln_o_b"], f32) @ W_o + np.asarray(inputs["b_o"], f32)
    gamma = np.asarray(inputs["gamma"], f32).copy()
    beta = np.asarray(inputs["beta"], f32).copy()
    gamma[0] /= G
    beta[0] /= G
    gamma[1] *= ASCALE
    beta[1] *= ASCALE
    gamma[3] /= N
    beta[3] /= N

    def lhsT8(w, kt):
        # [din, dout] -> [128, kt/2 pairs, 2, dout] fp8
        t = w.reshape(kt, 128, -1).transpose(1, 0, 2)  # [128, kt, dout]
        return np.ascontiguousarray(
            t.reshape(128, kt // 2, 2, t.shape[-1])).astype(fp8)

    def chan(v, ntiles):
        return np.ascontiguousarray(v.reshape(ntiles, 128).T).astype(f32)

    # diagonal conv stationaries: [128, NCH, 9, 2, 128] fp8
    dw_h = np.asarray(inputs["dw_h"], f32)
    dw_o = np.asarray(inputs["dw_o"], f32)
    dw_qk = np.asarray(inputs["dw_qk"], f32)
    diag = np.zeros((128, NCH, 9, 2, 128), f32)
    rng128 = np.arange(128)
    for ct in range(NCH):
        if ct < 16:
            wsrc = dw_h[:, ct * 128:(ct + 1) * 128]
        elif ct < 20:
            wsrc = dw_o[:, (ct - 16) * 128:(ct - 15) * 128]
        else:
            wsrc = dw_qk
        for pr, (k0, k1) in enumerate(PAIRS):
            diag[rng128, ct, pr, 0, rng128] = wsrc[k0]
            diag[rng128, ct, pr, 1, rng128] = wsrc[k1]
        diag[rng128, ct, 8, 0, rng128] = wsrc[16]
    return {
        "wh8": lhsT8(whp, 4), "wqk8": lhsT8(wqkp, 4), "wo8": lhsT8(wop, 8),
        "bh": chan(bhp, 16), "bqk": chan(bqkp, 1), "bo": chan(bop, 4),
        "gb": np.concatenate([gamma.T, beta.T], axis=1).astype(f32),
        "diag": diag.astype(fp8),
    }


_NC = None


def get_nc():
    global _NC
    if _NC is None:
        _NC = _build_nc()
    return _NC


def make_in_maps(inputs):
    x = np.asarray(inputs["x"], np.float32)
    B = x.shape[0]
    prep = prep_inputs(inputs)
    return [{"x": np.ascontiguousarray(x[b]), **prep} for b in range(B)]


def kernel(**inputs):
    nc = get_nc()
    in_maps = make_in_maps(inputs)
    res = bass_utils.run_bass_kernel_spmd(nc, in_maps, core_ids=list(range(8)))
    out = np.stack([res.results[b]["out"] for b in range(8)], axis=0)
    return out.astype(np.float32)


# revision 13
# speedup vs baseline: 1.4333x; 1.0949x over previous
"""Self-contained TRN2 kernel for nn_FLASH_ShareA_FFConvM_FlashAttn.

kernel(**inputs) takes the full (unsharded) inputs from setup_inputs() and
returns the full (B, N, D) float32 output. Internally: data-parallel over the
batch — one batch sample per NeuronCore, 8 cores, no collectives.

v2: all heavy matmuls in fp8 DoubleRow (paired k-tiles / paired conv taps),
depthwise convs fully on the PE as paired diagonal matmuls, attention weights
pre-scaled by 2^30 to stay in fp8 range, deferred output LayerNorm, and the
zspill round-trip replaced by an SBUF-resident transposed buffer.
"""
import sys

if "/opt/trn_rl_repo" not in sys.path:
    sys.path.insert(0, "/opt/trn_rl_repo")

import numpy as np
import ml_dtypes
import concourse.bass as bass
import concourse.bacc as bacc
import concourse.mybir as mybir
import concourse.tile as tile
from concourse import bass_utils
from concourse.ap import AP

F32 = mybir.dt.float32
BF16 = mybir.dt.bfloat16
FP8 = mybir.dt.float8e4
AF = mybir.ActivationFunctionType
OP = mybir.AluOpType
DR = mybir.MatmulPerfMode.DoubleRow

N, D, H, QK, G = 4096, 512, 2048, 128, 256
NG = N // G
NT = N // 128
KTAPS = 17
PAD = 8
NPADBUF = N + 2 * PAD  # fp8/bf16 padded conv input length (max tap read = N+15)
E2 = 2 * D
EPS = 1e-5
NCH = 21  # conv channel tiles: hid 0..15, out 16..19, qk 20
# conv tap pairs with stride-4 spacing (DR rows must be >=4 fp8 elements apart).
# Taps 8,12,9,13 run on the DVE instead (frees PE passes during P3/P5).
PAIRS = [(0, 4), (1, 5), (2, 6), (3, 7), (10, 14), (11, 15)]
DVETAPS = [8, 12, 9, 13]
ASCALE = float(2 ** 30)      # attention-weight scale kept inside psum
RELUSC = float(2 ** 15)      # sqrt(ASCALE), applied before squaring
GRP = 16                     # P4 deferred-LN batch size (iterations)


def _pair_ap(t, off, n):
    """Overlapping [128, 2, n] moving AP: row j reads t[:, off+4j : off+4j+n]."""
    base = t[:, 0:1]
    return AP(base.tensor, base.offset + off, [list(base.ap[0]), [4, 2], [1, n]])


def _emit_conv(nc, pool, dvp, dg, wv, h8t, hb, acc):
    """acc = h + conv(h): 6 DR tap pairs + tap16 on PE; 4 taps + identity on DVE."""
    # DVE partial: accd = h + sum_{k in DVETAPS} w_k * h_shift_k   (full width)
    accd = dvp.tile([128, N], BF16, tag="accd")
    tmp = dvp.tile([128, N], BF16, tag="dvtmp")
    for j, k in enumerate(DVETAPS):
        nc.vector.tensor_scalar(out=tmp[:, :], in0=hb[:, k:k + N],
                                scalar1=wv[:, j:j + 1], scalar2=None, op0=OP.mult)
        if j == 0:
            nc.vector.tensor_tensor(out=accd[:, :], in0=tmp[:, :],
                                    in1=hb[:, PAD:PAD + N], op=OP.add)
        else:
            nc.vector.tensor_tensor(out=accd[:, :], in0=tmp[:, :],
                                    in1=accd[:, :], op=OP.add)
        tmp = dvp.tile([128, N], BF16, tag="dvtmp")
    for c in range(8):
        cb = c * 512
        cp = pool.tile([128, 512], F32, tag="convps")
        for pr in range(6):
            nc.tensor.matmul(cp[:, :], dg[:, pr, :, :], _pair_ap(h8t, PAIRS[pr][0] + cb, 512),
                             start=(pr == 0), stop=False, perf_mode=DR)
        nc.tensor.matmul(cp[:, :], dg[:, 6, 0, :], h8t[:, 16 + cb:16 + cb + 512],
                         start=False, stop=True)
        nc.vector.tensor_tensor(out=acc[:, cb:cb + 512], in0=cp[:, :],
                                in1=accd[:, cb:cb + 512], op=OP.add)


def _emit(nc, tc, x, wh8, wqk8, wo8, bh, bqk, bo, gb, diag, dwv, out, spill):
    from contextlib import ExitStack
    es = ExitStack()
    consts = es.enter_context(tc.tile_pool(name="consts", bufs=1))
    wh_sb = consts.tile([128, 2, 2, H], FP8)
    nc.sync.dma_start(wh_sb[:, :, :, :], wh8.ap())
    wqk_sb = consts.tile([128, 2, 2, QK], FP8)
    nc.sync.dma_start(wqk_sb[:, :, :, :], wqk8.ap())
    wo_sb = consts.tile([128, 4, 2, D], FP8)
    nc.sync.dma_start(wo_sb[:, :, :, :], wo8.ap())
    bh_sb = consts.tile([128, 16], F32)
    nc.sync.dma_start(bh_sb[:, :], bh.ap())
    bqk_sb = consts.tile([128, 1], F32)
    nc.sync.dma_start(bqk_sb[:, :], bqk.ap())
    bo_sb = consts.tile([128, 4], F32)
    nc.sync.dma_start(bo_sb[:, :], bo.ap())
    gb_sb = consts.tile([128, 8], F32)
    nc.sync.dma_start(gb_sb[:, :], gb.ap())
    dwv_sb = consts.tile([128, NCH, 4], F32)
    nc.sync.dma_start(dwv_sb[:, :, :], dwv.ap())
    eps_sb = consts.tile([128, 1], F32)
    nc.vector.memset(eps_sb[:, :], EPS)

    outer = es.enter_context(tc.tile_pool(name="outer", bufs=1))
    attnT8 = outer.tile([128, NG, 2, G], FP8)
    lq_sb = outer.tile([128, N], BF16)
    lk_str = outer.tile([128, NT, 128], BF16)
    linkv_sb = outer.tile([128, E2], BF16)
    linku_sb = outer.tile([128, E2], BF16)
    sums = outer.tile([128, 32], F32)
    sumsq = outer.tile([128, 32], F32)
    qq_sb = outer.tile([128, N], BF16)
    qkk_sb = outer.tile([128, N], BF16)
    spill_v = spill.ap().rearrange("(tt p) (q c4) -> p tt q c4", p=128, c4=512)

    es2 = ExitStack()
    zpool = es2.enter_context(tc.tile_pool(name="zpool", bufs=1))
    zT8 = []
    for c in range(8):
        zT8c = zpool.tile([128, 4, 512], FP8, tag=f"zT8_{c}")
        zT8.append(zT8c)

    # ---------------- P0: token-shifted LayerNorm -> zT8 chunks ----------------
    with tc.tile_pool(name="p0", bufs=3) as p0, \
         tc.tile_pool(name="p0z", bufs=3) as p0z, \
         tc.tile_pool(name="p0s", bufs=4) as p0s:
        ztc = None
        for tt in range(NT):
            xt = p0.tile([128, D], F32, tag="xt")
            t0 = tt * 128
            if tt == 0:
                nc.vector.memset(xt[0:1, 0:D // 2], 0.0)
                nc.gpsimd.dma_start(xt[1:128, 0:D // 2], x[0:127, 0:D // 2])
            else:
                nc.gpsimd.dma_start(xt[:, 0:D // 2], x[t0 - 1:t0 + 127, 0:D // 2])
            nc.gpsimd.dma_start(xt[:, D // 2:D], x[t0:t0 + 128, D // 2:D])
            st6 = p0s.tile([128, 6], F32, tag="st6")
            nc.vector.bn_stats(st6[:, :], xt[:, :])
            mv = p0s.tile([128, 2], F32, tag="mv")
            nc.vector.bn_aggr(mv[:, :], st6[:, :])
            rstd = p0s.tile([128, 1], F32, tag="rstd")
            nc.scalar.activation(rstd[:, :], mv[:, 1:2], AF.Sqrt, bias=eps_sb[:, :], scale=1.0)
            nc.vector.reciprocal(rstd[:, :], rstd[:, :])
            nmu = p0s.tile([128, 1], F32, tag="nmu")
            nc.vector.tensor_scalar(out=nmu[:, :], in0=mv[:, 0:1], scalar1=rstd[:, :],
                                    scalar2=-1.0, op0=OP.mult, op1=OP.mult)
            zt = p0.tile([128, D], BF16, tag="zt")
            nc.scalar.activation(zt[:, :], xt[:, :], AF.Identity,
                                 bias=nmu[:, :], scale=rstd[:, :])
            if tt % 4 == 0:
                ztc = p0z.tile([128, 4, 512], BF16, tag="ztc")
            nc.sync.dma_start_transpose(ztc[:, :, (tt % 4) * 128:(tt % 4) * 128 + 128],
                                        zt[:, :])
            if tt % 4 == 3:
                nc.scalar.activation(zT8[tt // 4][:, :, :], ztc[:, :, :], AF.Copy)

    # ---------------- P1: qk path -> attnT8 / lq / lk_str ----------------
    with tc.tile_pool(name="p1", bufs=1) as p1, \
         tc.tile_pool(name="p1d", bufs=1) as p1d, \
         tc.tile_pool(name="p1v", bufs=2) as p1v, \
         tc.tile_pool(name="p1p", bufs=2, space="PSUM") as p1p, \
         tc.tile_pool(name="p1cp", bufs=3, space="PSUM") as p1cp:
        dgq = p1d.tile([128, 7, 2, 128], FP8, tag="dgq")
        nc.sync.dma_start(dgq[:, :, :, :], diag.ap()[:, 20, :, :, :])
        qkp = p1.tile([128, NPADBUF], BF16, tag="qkpad")
        nc.vector.memset(qkp[:, 0:PAD], 0.0)
        nc.vector.memset(qkp[:, PAD + N:], 0.0)
        q8p = p1.tile([128, NPADBUF], FP8, tag="qk8pad")
        for ch in range(8):
            ps = p1p.tile([128, 512], F32, tag="qkps")
            for pr in range(2):
                nc.tensor.matmul(ps[:, :], wqk_sb[:, pr, :, :],
                                 zT8[ch][:, 2 * pr:2 * pr + 2, :],
                                 start=(pr == 0), stop=(pr == 1), perf_mode=DR)
            nc.scalar.activation(qkp[:, PAD + ch * 512:PAD + (ch + 1) * 512], ps[:, :],
                                 AF.Silu, bias=bqk_sb[:, :], scale=1.0)
        nc.scalar.activation(q8p[:, :], qkp[:, :], AF.Copy)
        qkc = p1.tile([128, N], BF16, tag="qkc")
        _emit_conv(nc, p1cp, p1v, dgq, dwv_sb[:, 20, :], q8p, qkp, qkc)
        lkk = p1.tile([128, N], BF16, tag="lkk")
        for i, dst in ((0, qq_sb), (1, lq_sb), (2, qkk_sb), (3, lkk)):
            nc.vector.tensor_scalar(out=dst[:, :], in0=qkc[:, :], scalar1=gb_sb[:, i:i + 1],
                                    scalar2=gb_sb[:, 4 + i:5 + i], op0=OP.mult, op1=OP.add)
        nc.sync.dma_start_transpose(lk_str[:, :, :], lkk[:, :])

    # ---------------- P3: hidden FFConvM -> spill + lin_kv/lin_ku ----------------
    with tc.tile_pool(name="p3", bufs=2) as p3, \
         tc.tile_pool(name="p3d", bufs=3) as p3d, \
         tc.tile_pool(name="p3q", bufs=1) as p3q, \
         tc.tile_pool(name="p3v", bufs=2) as p3v, \
         tc.tile_pool(name="p3p", bufs=2, space="PSUM") as p3p, \
         tc.tile_pool(name="p3cp", bufs=3, space="PSUM") as p3cp, \
         tc.tile_pool(name="p3lin", bufs=1, space="PSUM") as p3lin:
        state = {"strips4": None}

        def produce(hc):
            dg = p3d.tile([128, 7, 2, 128], FP8, tag="dg")
            nc.sync.dma_start(dg[:, :, :, :], diag.ap()[:, hc, :, :, :])
            hb = p3.tile([128, NPADBUF], BF16, tag="hpad")
            nc.vector.memset(hb[:, 0:PAD], 0.0)
            nc.vector.memset(hb[:, PAD + N:], 0.0)
            for cp2 in range(4):
                for k in range(2):
                    c = 2 * cp2 + k
                    ps = p3p.tile([128, 512], F32, tag="hps")
                    for pr in range(2):
                        nc.tensor.matmul(ps[:, :], wh_sb[:, pr, :, hc * 128:(hc + 1) * 128],
                                         zT8[c][:, 2 * pr:2 * pr + 2, :],
                                         start=(pr == 0), stop=(pr == 1), perf_mode=DR)
                    nc.scalar.activation(hb[:, PAD + c * 512:PAD + (c + 1) * 512], ps[:, :],
                                         AF.Silu, bias=bh_sb[:, hc:hc + 1], scale=1.0)
            h8 = p3.tile([128, NPADBUF], FP8, tag="h8pad")
            nc.scalar.activation(h8[:, :], hb[:, :], AF.Copy)
            return dg, hb, h8

        def convpost(hc, dg, hb, h8):
            if hc % 4 == 0:
                s4_new = p3q.tile([128, NT, 4, 128], BF16, tag="strips4")
                state["strips4"] = s4_new
            strips4 = state["strips4"]
            acc = p3.tile([128, N], BF16, tag="acc")
            _emit_conv(nc, p3cp, p3v, dg, dwv_sb[:, hc, :], h8, hb, acc)
            nc.sync.dma_start_transpose(strips4[:, :, hc % 4, :], acc[:, :])
            if hc % 4 == 3:
                q = hc // 4
                nc.gpsimd.dma_start(spill_v[:, :, q, :], strips4[:, :, :, :])
                kvp = p3lin.tile([128, 512], F32, tag="kvps")
                for tt in range(NT):
                    nc.tensor.matmul(
                        kvp[:, :], lk_str[:, tt, :],
                        strips4[:, tt, :, :].rearrange("p a c -> p (a c)"),
                        start=(tt == 0), stop=(tt == NT - 1))
                dst = linkv_sb if q < 2 else linku_sb
                nc.scalar.activation(dst[:, (q % 2) * 512:(q % 2) * 512 + 512],
                                     kvp[:, :], AF.Copy)

        prev = None
        for hc in range(16):
            pr = produce(hc)
            if hc == 0:
                # attention scores: emitted here so the PE's qk-conv wait
                # overlaps the first hidden GEMM.
                with tc.tile_pool(name="p1s", bufs=3) as p1s, \
                     tc.tile_pool(name="p1sp", bufs=2, space="PSUM") as p1sp:
                    for g in range(NG):
                        for jh in range(2):
                            sp = p1sp.tile([128, G], F32, tag="simps")
                            nc.tensor.matmul(sp[:, :],
                                             qkk_sb[:, g * G + jh * 128: g * G + jh * 128 + 128],
                                             qq_sb[:, g * G:(g + 1) * G],
                                             start=True, stop=True)
                            rel = p1s.tile([128, G], BF16, tag="rel")
                            nc.scalar.activation(rel[:, :], sp[:, :], AF.Relu, scale=RELUSC)
                            nc.scalar.activation(attnT8[:, g, jh, :], rel[:, :], AF.Square)
            if prev is not None:
                convpost(*prev)
            prev = (hc, *pr)
        convpost(*prev)

    es2.close()  # frees zT8 chunks before the P4/P5 pools open

    # ---------------- P4: attention apply + gating (deferred LN) ----------------
    with tc.tile_pool(name="mid", bufs=1) as mid:
        zoT8 = mid.tile([128, 8, N], FP8)
        vo_big = mid.tile([128, NT, 4, 128], BF16)
        with tc.tile_pool(name="p4", bufs=2) as p4, \
             tc.tile_pool(name="p4g", bufs=GRP + 2) as p4g, \
             tc.tile_pool(name="p4s", bufs=2) as p4s, \
             tc.tile_pool(name="p4p", bufs=2, space="PSUM") as p4p:
            govu = []   # (go, vt?, ...) per pending it in current group
            for g in range(NG):
                vt, ut = [], []
                for jh in range(2):
                    vtj = p4.tile([128, E2], BF16, tag=f"vg{jh}")
                    nc.gpsimd.dma_start(vtj[:, :], spill[g * G + jh * 128: g * G + jh * 128 + 128, 0:E2])
                    utj = p4.tile([128, E2], BF16, tag=f"ug{jh}")
                    nc.gpsimd.dma_start(utj[:, :], spill[g * G + jh * 128: g * G + jh * 128 + 128, E2:H])
                    vt.append(vtj)
                    ut.append(utj)
                vt8 = p4.tile([128, 2, 2, 512], FP8, tag="vt8")
                ut8 = p4.tile([128, 2, 2, 512], FP8, tag="ut8")
                for jh in range(2):
                    for e in range(2):
                        nc.vector.tensor_copy(vt8[:, e, jh, :], vt[jh][:, e * 512:(e + 1) * 512])
                        nc.scalar.activation(ut8[:, e, jh, :], ut[jh][:, e * 512:(e + 1) * 512],
                                             AF.Copy)
                for it in range(2):
                    idx = g * 2 + it
                    islice = slice(g * G + it * 128, g * G + it * 128 + 128)
                    avp = p4p.tile([128, E2], F32, tag="avps")
                    aup = p4p.tile([128, E2], F32, tag="aups")
                    for dst, m8, lin in ((avp, vt8, linkv_sb), (aup, ut8, linku_sb)):
                        for e in range(2):
                            nc.tensor.matmul(dst[:, e * 512:(e + 1) * 512],
                                             attnT8[:, g, :, it * 128:it * 128 + 128],
                                             m8[:, e, :, :],
                                             start=True, stop=False, perf_mode=DR)
                            nc.tensor.matmul(dst[:, e * 512:(e + 1) * 512],
                                             lq_sb[:, islice], lin[:, e * 512:(e + 1) * 512],
                                             start=False, stop=True)
                    t1 = p4s.tile([128, E2], BF16, tag="t1")
                    nc.vector.scalar_tensor_tensor(out=t1[:, :], in0=avp[:, :],
                                                   scalar=1.0 / ASCALE, in1=ut[it][:, :],
                                                   op0=OP.mult, op1=OP.mult)
                    sg = p4s.tile([128, E2], BF16, tag="sg")
                    nc.scalar.activation(sg[:, :], t1[:, :], AF.Sigmoid)
                    t2 = p4s.tile([128, E2], BF16, tag="t2")
                    nc.vector.scalar_tensor_tensor(out=t2[:, :], in0=aup[:, :],
                                                   scalar=1.0 / ASCALE, in1=vt[it][:, :],
                                                   op0=OP.mult, op1=OP.mult)
                    go = p4g.tile([128, E2], BF16, tag="go")
                    nc.vector.scalar_tensor_tensor(out=go[:, :], in0=t2[:, :], scalar=1.0,
                                                   in1=sg[:, :], op0=OP.mult, op1=OP.mult,
                                                   accum_out=sums[:, idx:idx + 1])
                    jnk = p4s.tile([128, E2], BF16, tag="jnk")
                    nc.scalar.activation(jnk[:, :], go[:, :], AF.Square,
                                         accum_out=sumsq[:, idx:idx + 1])
                    govu.append(go)
                    if len(govu) == GRP:
                        _p4_norm(nc, tc, p4s, govu, sums, sumsq, eps_sb, zoT8,
                                 idx - GRP + 1)
                        govu = []

        # ---------------- P5: output FFConvM ----------------
        with tc.tile_pool(name="p5", bufs=2) as p5, \
             tc.tile_pool(name="p5d", bufs=2) as p5d, \
             tc.tile_pool(name="p5v", bufs=2) as p5v, \
             tc.tile_pool(name="p5p", bufs=2, space="PSUM") as p5p, \
             tc.tile_pool(name="p5cp", bufs=3, space="PSUM") as p5cp:
            def produce5(oc):
                dg = p5d.tile([128, 7, 2, 128], FP8, tag="dg5")
                nc.sync.dma_start(dg[:, :, :, :], diag.ap()[:, 16 + oc, :, :, :])
                hb = p5.tile([128, NPADBUF], BF16, tag="hpad5")
                nc.vector.memset(hb[:, 0:PAD], 0.0)
                nc.vector.memset(hb[:, PAD + N:], 0.0)
                for c in range(8):
                    ps = p5p.tile([128, 512], F32, tag="ops")
                    for pr in range(4):
                        nc.tensor.matmul(ps[:, :], wo_sb[:, pr, :, oc * 128:(oc + 1) * 128],
                                         zoT8[:, 2 * pr:2 * pr + 2, c * 512:(c + 1) * 512],
                                         start=(pr == 0), stop=(pr == 3), perf_mode=DR)
                    nc.scalar.activation(hb[:, PAD + c * 512:PAD + (c + 1) * 512], ps[:, :],
                                         AF.Silu, bias=bo_sb[:, oc:oc + 1], scale=1.0)
                h8 = p5.tile([128, NPADBUF], FP8, tag="h85")
                nc.scalar.activation(h8[:, :], hb[:, :], AF.Copy)
                return dg, hb, h8

            def convpost5(oc, dg, hb, h8):
                acc = p5.tile([128, N], BF16, tag="acc5")
                _emit_conv(nc, p5cp, p5v, dg, dwv_sb[:, 16 + oc, :], h8, hb, acc)
                nc.sync.dma_start_transpose(vo_big[:, :, oc, :], acc[:, :])

            prev = None
            for oc in range(4):
                pr = produce5(oc)
                if prev is not None:
                    convpost5(*prev)
                prev = (oc, *pr)
            convpost5(*prev)

        # ---------------- P6: residual ----------------
        with tc.tile_pool(name="p6", bufs=3) as p6:
            for tt in range(NT):
                xt = p6.tile([128, D], F32, tag="xt6")
                nc.gpsimd.dma_start(xt[:, :], x[tt * 128:(tt + 1) * 128, :])
                of = p6.tile([128, D], F32, tag="of")
                nc.vector.tensor_tensor(out=of[:, :], in0=xt[:, :],
                                        in1=vo_big[:, tt, :, :].rearrange("p a c -> p (a c)"),
                                        op=OP.add)
                nc.gpsimd.dma_start(out[tt * 128:(tt + 1) * 128, :], of[:, :])
    es.close()


def _p4_norm(nc, tc, pool, gos, sums, sumsq, eps_sb, zoT8, idx0):
    """Deferred LayerNorm for GRP gating tiles: batched stats then per-tile
    normalize + transpose + fp8 convert."""
    n = len(gos)
    sl = slice(idx0, idx0 + n)
    mean = pool.tile([128, n], F32, tag="mean")
    nc.vector.tensor_scalar(out=mean[:, :], in0=sums[:, sl], scalar1=1.0 / E2,
                            scalar2=None, op0=OP.mult)
    msq = pool.tile([128, n], F32, tag="msq")
    nc.vector.tensor_tensor(out=msq[:, :], in0=mean[:, :], in1=mean[:, :], op=OP.mult)
    var = pool.tile([128, n], F32, tag="var")
    nc.vector.scalar_tensor_tensor(out=var[:, :], in0=sumsq[:, sl], scalar=1.0 / E2,
                                   in1=msq[:, :], op0=OP.mult, op1=OP.subtract)
    rstd = pool.tile([128, n], F32, tag="rstdn")
    nc.scalar.activation(rstd[:, :], var[:, :], AF.Sqrt, bias=eps_sb[:, :], scale=1.0)
    nc.vector.reciprocal(rstd[:, :], rstd[:, :])
    nmu = pool.tile([128, n], F32, tag="nmun")
    nc.vector.tensor_tensor(out=nmu[:, :], in0=mean[:, :], in1=rstd[:, :], op=OP.mult)
    nc.vector.tensor_scalar(out=nmu[:, :], in0=nmu[:, :], scalar1=-1.0,
                            scalar2=None, op0=OP.mult)
    for j, go in enumerate(gos):
        tti = idx0 + j
        zo = pool.tile([128, E2], BF16, tag="zon")
        nc.scalar.activation(zo[:, :], go[:, :], AF.Identity,
                             bias=nmu[:, j:j + 1], scale=rstd[:, j:j + 1])
        zot = pool.tile([128, 8, 128], BF16, tag="zot")
        nc.sync.dma_start_transpose(zot[:, :, :], zo[:, :])
        nc.scalar.activation(zoT8[:, :, tti * 128:(tti + 1) * 128], zot[:, :, :], AF.Copy)


def _build_nc():
    nc = bacc.Bacc("TRN2", target_bir_lowering=False, debug=False)
    x = nc.dram_tensor("x", [N, D], F32, kind="ExternalInput")
    wh8 = nc.dram_tensor("wh8", [128, 2, 2, H], FP8, kind="ExternalInput")
    wqk8 = nc.dram_tensor("wqk8", [128, 2, 2, QK], FP8, kind="ExternalInput")
    wo8 = nc.dram_tensor("wo8", [128, 4, 2, D], FP8, kind="ExternalInput")
    bh = nc.dram_tensor("bh", [128, 16], F32, kind="ExternalInput")
    bqk = nc.dram_tensor("bqk", [128, 1], F32, kind="ExternalInput")
    bo = nc.dram_tensor("bo", [128, 4], F32, kind="ExternalInput")
    gb = nc.dram_tensor("gb", [128, 8], F32, kind="ExternalInput")
    diag = nc.dram_tensor("diag", [128, NCH, 7, 2, 128], FP8, kind="ExternalInput")
    dwv = nc.dram_tensor("dwv", [128, NCH, 4], F32, kind="ExternalInput")
    out = nc.dram_tensor("out", [N, D], F32, kind="ExternalOutput")
    spill = nc.dram_tensor("spill", [N, H], BF16)
    with tile.TileContext(nc) as tc:
        _emit(nc, tc, x, wh8, wqk8, wo8, bh, bqk, bo, gb, diag, dwv, out, spill)
    nc.compile()
    return nc


def prep_inputs(inputs):
    f32 = np.float32
    fp8 = ml_dtypes.float8_e4m3
    W_h = np.asarray(inputs["W_h"], f32)
    W_qk = np.asarray(inputs["W_qk"], f32)
    W_o = np.asarray(inputs["W_o"], f32)
    whp = np.asarray(inputs["ln_h_g"], f32)[:, None] * W_h
    bhp = np.asarray(inputs["ln_h_b"], f32) @ W_h + np.asarray(inputs["b_h"], f32)
    wqkp = np.asarray(inputs["ln_qk_g"], f32)[:, None] * W_qk
    bqkp = np.asarray(inputs["ln_qk_b"], f32) @ W_qk + np.asarray(inputs["b_qk"], f32)
    wop = np.asarray(inputs["ln_o_g"], f32)[:, None] * W_o
    bop = np.asarray(inputs["ln_o_b"], f32) @ W_o + np.asarray(inputs["b_o"], f32)
    gamma = np.asarray(inputs["gamma"], f32).copy()
    beta = np.asarray(inputs["beta"], f32).copy()
    gamma[0] /= G
    beta[0] /= G
    gamma[1] *= ASCALE
    beta[1] *= ASCALE
    gamma[3] /= N
    beta[3] /= N

    def lhsT8(w, kt):
        # [din, dout] -> [128, kt/2 pairs, 2, dout] fp8
        t = w.reshape(kt, 128, -1).transpose(1, 0, 2)  # [128, kt, dout]
        return np.ascontiguousarray(
            t.reshape(128, kt // 2, 2, t.shape[-1])).astype(fp8)

    def chan(v, ntiles):
        return np.ascontiguousarray(v.reshape(ntiles, 128).T).astype(f32)

    # diagonal conv stationaries: [128, NCH, 9, 2, 128] fp8
    dw_h = np.asarray(inputs["dw_h"], f32)
    dw_o = np.asarray(inputs["dw_o"], f32)
    dw_qk = np.asarray(inputs["dw_qk"], f32)
    diag = np.zeros((128, NCH, 7, 2, 128), f32)
    dwv = np.zeros((128, NCH, 4), f32)
    rng128 = np.arange(128)
    for ct in range(NCH):
        if ct < 16:
            wsrc = dw_h[:, ct * 128:(ct + 1) * 128]
        elif ct < 20:
            wsrc = dw_o[:, (ct - 16) * 128:(ct - 15) * 128]
        else:
            wsrc = dw_qk
        for pr, (k0, k1) in enumerate(PAIRS):
            diag[rng128, ct, pr, 0, rng128] = wsrc[k0]
            diag[rng128, ct, pr, 1, rng128] = wsrc[k1]
        diag[rng128, ct, 6, 0, rng128] = wsrc[16]
        for j, k in enumerate(DVETAPS):
            dwv[:, ct, j] = wsrc[k]
    return {
        "wh8": lhsT8(whp, 4), "wqk8": lhsT8(wqkp, 4), "wo8": lhsT8(wop, 8),
        "bh": chan(bhp, 16), "bqk": chan(bqkp, 1), "bo": chan(bop, 4),
        "gb": np.concatenate([gamma.T, beta.T], axis=1).astype(f32),
        "diag": diag.astype(fp8), "dwv": dwv,
    }


_NC = None


def get_nc():
    global _NC
    if _NC is None:
        _NC = _build_nc()
    return _NC


def make_in_maps(inputs):
    x = np.asarray(inputs["x"], np.float32)
    B = x.shape[0]
    prep = prep_inputs(inputs)
    return [{"x": np.ascontiguousarray(x[b]), **prep} for b in range(B)]


def kernel(**inputs):
    nc = get_nc()
    in_maps = make_in_maps(inputs)
    res = bass_utils.run_bass_kernel_spmd(nc, in_maps, core_ids=list(range(8)))
    out = np.stack([res.results[b]["out"] for b in range(8)], axis=0)
    return out.astype(np.float32)


# revision 18
# speedup vs baseline: 1.6119x; 1.1246x over previous
"""Self-contained TRN2 kernel for nn_FLASH_ShareA_FFConvM_FlashAttn.

kernel(**inputs) takes the full (unsharded) inputs from setup_inputs() and
returns the full (B, N, D) float32 output. Internally: data-parallel over the
batch — one batch sample per NeuronCore, 8 cores, no collectives.

v2: all heavy matmuls in fp8 DoubleRow (paired k-tiles / paired conv taps),
depthwise convs fully on the PE as paired diagonal matmuls, attention weights
pre-scaled by 2^30 to stay in fp8 range, deferred output LayerNorm, and the
zspill round-trip replaced by an SBUF-resident transposed buffer.
"""
import sys

if "/opt/trn_rl_repo" not in sys.path:
    sys.path.insert(0, "/opt/trn_rl_repo")

import numpy as np
import ml_dtypes
import concourse.bass as bass
import concourse.bacc as bacc
import concourse.mybir as mybir
import concourse.tile as tile
from concourse import bass_utils
from concourse.ap import AP

F32 = mybir.dt.float32
BF16 = mybir.dt.bfloat16
FP8 = mybir.dt.float8e4
AF = mybir.ActivationFunctionType
OP = mybir.AluOpType
DR = mybir.MatmulPerfMode.DoubleRow

N, D, H, QK, G = 4096, 512, 2048, 128, 256
NG = N // G
NT = N // 128
KTAPS = 17
PAD = 8
NPADBUF = N + 2 * PAD  # fp8/bf16 padded conv input length (max tap read = N+15)
E2 = 2 * D
EPS = 1e-5
NCH = 21  # conv channel tiles: hid 0..15, out 16..19, qk 20
# conv tap pairs with stride-4 spacing (DR rows must be >=4 fp8 elements apart).
# Taps 8,12,9,13 run on the DVE instead (frees PE passes during P3/P5).
PAIRS = [(0, 4), (1, 5), (2, 6), (3, 7), (10, 14), (11, 15)]
DVETAPS = [8, 12, 9, 13]
ASCALE = float(2 ** 30)      # attention-weight scale kept inside psum
RELUSC = float(2 ** 15)      # sqrt(ASCALE), applied before squaring
GRP = 16                     # P4 deferred-LN batch size (iterations)


def _pair_ap(t, off, n):
    """Overlapping [128, 2, n] moving AP: row j reads t[:, off+4j : off+4j+n]."""
    base = t[:, 0:1]
    return AP(base.tensor, base.offset + off, [list(base.ap[0]), [4, 2], [1, n]])


def _emit_conv(nc, pool, dvp, dg, wv, h8t, hb, acc):
    """acc = h + conv(h): 6 DR tap pairs + tap16 on PE; 4 taps + identity on DVE."""
    # DVE partial: accd = h + sum_{k in DVETAPS} w_k * h_shift_k   (full width)
    accd = dvp.tile([128, N], BF16, tag="accd")
    tmp = dvp.tile([128, N], BF16, tag="dvtmp")
    for j, k in enumerate(DVETAPS):
        nc.vector.tensor_scalar(out=tmp[:, :], in0=hb[:, k:k + N],
                                scalar1=wv[:, j:j + 1], scalar2=None, op0=OP.mult)
        if j == 0:
            nc.vector.tensor_tensor(out=accd[:, :], in0=tmp[:, :],
                                    in1=hb[:, PAD:PAD + N], op=OP.add)
        else:
            nc.vector.tensor_tensor(out=accd[:, :], in0=tmp[:, :],
                                    in1=accd[:, :], op=OP.add)
    for c in range(8):
        cb = c * 512
        cp = pool.tile([128, 512], F32, tag="convps")
        for pr in range(6):
            nc.tensor.matmul(cp[:, :], dg[:, pr, :, :], _pair_ap(h8t, PAIRS[pr][0] + cb, 512),
                             start=(pr == 0), stop=False, perf_mode=DR)
        nc.tensor.matmul(cp[:, :], dg[:, 6, 0, :], h8t[:, 16 + cb:16 + cb + 512],
                         start=False, stop=True)
        nc.vector.tensor_tensor(out=acc[:, cb:cb + 512], in0=cp[:, :],
                                in1=accd[:, cb:cb + 512], op=OP.add)


def _emit(nc, tc, x, wh8, wqk8, wo8, bh, bqk, bo, gb, diag, dwv, out, spill):
    from contextlib import ExitStack
    es = ExitStack()
    consts = es.enter_context(tc.tile_pool(name="consts", bufs=1))
    wh_sb = consts.tile([128, 2, 2, H], FP8)
    nc.sync.dma_start(wh_sb[:, :, :, :], wh8.ap())
    wqk_sb = consts.tile([128, 2, 2, QK], FP8)
    nc.sync.dma_start(wqk_sb[:, :, :, :], wqk8.ap())
    wo_sb = consts.tile([128, 4, 2, D], FP8)
    nc.sync.dma_start(wo_sb[:, :, :, :], wo8.ap())
    bh_sb = consts.tile([128, 16], F32)
    nc.sync.dma_start(bh_sb[:, :], bh.ap())
    bqk_sb = consts.tile([128, 1], F32)
    nc.sync.dma_start(bqk_sb[:, :], bqk.ap())
    bo_sb = consts.tile([128, 4], F32)
    nc.sync.dma_start(bo_sb[:, :], bo.ap())
    gb_sb = consts.tile([128, 8], F32)
    nc.sync.dma_start(gb_sb[:, :], gb.ap())
    dwv_sb = consts.tile([128, NCH, 4], F32)
    nc.sync.dma_start(dwv_sb[:, :, :], dwv.ap())
    eps_sb = consts.tile([128, 1], F32)
    nc.vector.memset(eps_sb[:, :], EPS)

    outer = es.enter_context(tc.tile_pool(name="outer", bufs=1))
    attnT8 = outer.tile([128, NG, 2, G], FP8)
    lq_sb = outer.tile([128, N], BF16)
    lk_str = outer.tile([128, NT, 128], BF16)
    linkv_sb = outer.tile([128, E2], BF16)
    linku_sb = outer.tile([128, E2], BF16)
    sums = outer.tile([128, 32], F32)
    sumsq = outer.tile([128, 32], F32)
    qq_sb = outer.tile([128, N], BF16)
    qkk_sb = outer.tile([128, N], BF16)
    spill_v = spill.ap().rearrange("(tt p) (q c4) -> p tt q c4", p=128, c4=512)

    es2 = ExitStack()
    zpool = es2.enter_context(tc.tile_pool(name="zpool", bufs=1))
    zT8 = []
    for c in range(8):
        zT8c = zpool.tile([128, 4, 512], FP8, tag=f"zT8_{c}")
        zT8.append(zT8c)

    # ---------------- P0: token-shifted LayerNorm -> zT8 chunks ----------------
    # x loads batched 4 token-tiles per DMA; shifted first-half channels loaded
    # separately with a one-row offset.
    xs_v = x.ap().rearrange("(c p) d -> p c d", p=128)
    with tc.tile_pool(name="p0", bufs=3) as p0, \
         tc.tile_pool(name="p0z", bufs=3) as p0z, \
         tc.tile_pool(name="p0s", bufs=6) as p0s:
        for cch in range(8):
            x4 = p0.tile([128, 4, D], F32, tag="x4")
            t0 = cch * 512
            if cch == 0:
                nc.vector.memset(x4[0:1, 0, 0:D // 2], 0.0)
                nc.gpsimd.dma_start(x4[1:128, 0, 0:D // 2], x[0:127, 0:D // 2])
                for j in range(1, 4):
                    nc.gpsimd.dma_start(x4[:, j, 0:D // 2],
                                        x[t0 + j * 128 - 1:t0 + j * 128 + 127, 0:D // 2])
            else:
                nc.gpsimd.dma_start(
                    x4[:, :, 0:D // 2],
                    x.ap()[t0 - 1:t0 + 511, 0:D // 2].rearrange("(j p) d -> p j d", p=128))
            nc.gpsimd.dma_start(x4[:, :, D // 2:D],
                                xs_v[:, 4 * cch:4 * cch + 4, D // 2:D])
            ztc = p0z.tile([128, 4, 512], BF16, tag="ztc")
            for j in range(4):
                st6 = p0s.tile([128, 6], F32, tag="st6")
                nc.vector.bn_stats(st6[:, :], x4[:, j, :])
                mv = p0s.tile([128, 2], F32, tag="mv")
                nc.vector.bn_aggr(mv[:, :], st6[:, :])
                rstd = p0s.tile([128, 1], F32, tag="rstd")
                nc.scalar.activation(rstd[:, :], mv[:, 1:2], AF.Sqrt, bias=eps_sb[:, :], scale=1.0)
                nc.vector.reciprocal(rstd[:, :], rstd[:, :])
                nmu = p0s.tile([128, 1], F32, tag="nmu")
                nc.vector.tensor_scalar(out=nmu[:, :], in0=mv[:, 0:1], scalar1=rstd[:, :],
                                        scalar2=-1.0, op0=OP.mult, op1=OP.mult)
                zt = p0.tile([128, D], BF16, tag="zt")
                nc.scalar.activation(zt[:, :], x4[:, j, :], AF.Identity,
                                     bias=nmu[:, :], scale=rstd[:, :])
                nc.sync.dma_start_transpose(ztc[:, :, j * 128:j * 128 + 128], zt[:, :])
            nc.scalar.activation(zT8[cch][:, :, :], ztc[:, :, :], AF.Copy)

    # ---------------- P1+P3: qk path interleaved with hidden FFConvM ----------------
    with tc.tile_pool(name="p1", bufs=1) as p1, \
         tc.tile_pool(name="p3", bufs=2) as p3, \
         tc.tile_pool(name="p3d", bufs=3) as p3d, \
         tc.tile_pool(name="p3q", bufs=1) as p3q, \
         tc.tile_pool(name="p3v", bufs=2) as p3v, \
         tc.tile_pool(name="p1s", bufs=3) as p1s, \
         tc.tile_pool(name="p3p", bufs=2, space="PSUM") as p3p, \
         tc.tile_pool(name="p3cp", bufs=3, space="PSUM") as p3cp, \
         tc.tile_pool(name="p1sp", bufs=2, space="PSUM") as p1sp, \
         tc.tile_pool(name="p3lin", bufs=1, space="PSUM") as p3lin:
        state = {"strips4": None}

        def produce(hc):
            dg = p3d.tile([128, 7, 2, 128], FP8, tag="dg")
            nc.sync.dma_start(dg[:, :, :, :], diag.ap()[:, hc, :, :, :])
            hb = p3.tile([128, NPADBUF], BF16, tag="hpad")
            nc.vector.memset(hb[:, 0:PAD], 0.0)
            nc.vector.memset(hb[:, PAD + N:], 0.0)
            for cp2 in range(4):
                for k in range(2):
                    c = 2 * cp2 + k
                    ps = p3p.tile([128, 512], F32, tag="hps")
                    for pr in range(2):
                        nc.tensor.matmul(ps[:, :], wh_sb[:, pr, :, hc * 128:(hc + 1) * 128],
                                         zT8[c][:, 2 * pr:2 * pr + 2, :],
                                         start=(pr == 0), stop=(pr == 1), perf_mode=DR)
                    nc.scalar.activation(hb[:, PAD + c * 512:PAD + (c + 1) * 512], ps[:, :],
                                         AF.Silu, bias=bh_sb[:, hc:hc + 1], scale=1.0)
            h8 = p3.tile([128, NPADBUF], FP8, tag="h8pad")
            nc.scalar.activation(h8[:, :], hb[:, :], AF.Copy)
            return dg, hb, h8

        def convpost(hc, dg, hb, h8):
            if hc % 4 == 0:
                s4_new = p3q.tile([128, NT, 4, 128], BF16, tag="strips4")
                state["strips4"] = s4_new
            strips4 = state["strips4"]
            acc = p3.tile([128, N], BF16, tag="acc")
            _emit_conv(nc, p3cp, p3v, dg, dwv_sb[:, hc, :], h8, hb, acc)
            nc.sync.dma_start_transpose(strips4[:, :, hc % 4, :], acc[:, :])
            if hc % 4 == 3:
                q = hc // 4
                nc.gpsimd.dma_start(spill_v[:, :, q, :], strips4[:, :, :, :])
                kvp = p3lin.tile([128, 512], F32, tag="kvps")
                for tt in range(NT):
                    nc.tensor.matmul(
                        kvp[:, :], lk_str[:, tt, :],
                        strips4[:, tt, :, :].rearrange("p a c -> p (a c)"),
                        start=(tt == 0), stop=(tt == NT - 1))
                dst = linkv_sb if q < 2 else linku_sb
                nc.scalar.activation(dst[:, (q % 2) * 512:(q % 2) * 512 + 512],
                                     kvp[:, :], AF.Copy)

        # qk GEMM + silu first (PE then Act); hidden produce(0) overlaps the
        # Act-side qk chain; qk conv runs on PE behind produce(0)'s GEMMs.
        dgq = p3d.tile([128, 7, 2, 128], FP8, tag="dg")
        nc.sync.dma_start(dgq[:, :, :, :], diag.ap()[:, 20, :, :, :])
        qkp = p1.tile([128, NPADBUF], BF16, tag="qkpad")
        nc.vector.memset(qkp[:, 0:PAD], 0.0)
        nc.vector.memset(qkp[:, PAD + N:], 0.0)
        q8p = p1.tile([128, NPADBUF], FP8, tag="qk8pad")
        for ch in range(8):
            ps = p3p.tile([128, 512], F32, tag="hps")
            for pr in range(2):
                nc.tensor.matmul(ps[:, :], wqk_sb[:, pr, :, :],
                                 zT8[ch][:, 2 * pr:2 * pr + 2, :],
                                 start=(pr == 0), stop=(pr == 1), perf_mode=DR)
            nc.scalar.activation(qkp[:, PAD + ch * 512:PAD + (ch + 1) * 512], ps[:, :],
                                 AF.Silu, bias=bqk_sb[:, :], scale=1.0)
        nc.scalar.activation(q8p[:, :], qkp[:, :], AF.Copy)
        pr0 = produce(0)
        qkc = p3.tile([128, N], BF16, tag="acc")
        _emit_conv(nc, p3cp, p3v, dgq, dwv_sb[:, 20, :], q8p, qkp, qkc)
        lkk = p3.tile([128, N], BF16, tag="acc")
        for i, dst in ((0, qq_sb), (1, lq_sb), (2, qkk_sb), (3, lkk)):
            nc.vector.tensor_scalar(out=dst[:, :], in0=qkc[:, :], scalar1=gb_sb[:, i:i + 1],
                                    scalar2=gb_sb[:, 4 + i:5 + i], op0=OP.mult, op1=OP.add)
        nc.sync.dma_start_transpose(lk_str[:, :, :], lkk[:, :])

        prev = (0, *pr0)
        for hc in range(1, 16):
            pr = produce(hc)
            if hc == 1:
                # attention scores: PE work that fills the gap while the qk
                # conv's DVE/Act chain finishes.
                for g in range(NG):
                    for jh in range(2):
                        sp = p1sp.tile([128, G], F32, tag="simps")
                        nc.tensor.matmul(sp[:, :],
                                         qkk_sb[:, g * G + jh * 128: g * G + jh * 128 + 128],
                                         qq_sb[:, g * G:(g + 1) * G],
                                         start=True, stop=True)
                        rel = p1s.tile([128, G], BF16, tag="rel")
                        nc.scalar.activation(rel[:, :], sp[:, :], AF.Relu, scale=RELUSC)
                        nc.scalar.activation(attnT8[:, g, jh, :], rel[:, :], AF.Square)
            convpost(*prev)
            prev = (hc, *pr)
        convpost(*prev)

    es2.close()  # frees zT8 chunks before the P4/P5 pools open

    # ---------------- P4: attention apply + gating (deferred LN) ----------------
    with tc.tile_pool(name="mid", bufs=1) as mid:
        zoT8 = mid.tile([128, 8, N], FP8)
        vo_big = mid.tile([128, NT, 4, 128], BF16)
        with tc.tile_pool(name="p4", bufs=2) as p4, \
             tc.tile_pool(name="p4g", bufs=GRP + 2) as p4g, \
             tc.tile_pool(name="p4s", bufs=2) as p4s, \
             tc.tile_pool(name="p4p", bufs=2, space="PSUM") as p4p:
            govu = []   # (go, vt?, ...) per pending it in current group
            for g in range(NG):
                vt, ut = [], []
                for jh in range(2):
                    vtj = p4.tile([128, E2], BF16, tag=f"vg{jh}")
                    nc.gpsimd.dma_start(vtj[:, :], spill[g * G + jh * 128: g * G + jh * 128 + 128, 0:E2])
                    utj = p4.tile([128, E2], BF16, tag=f"ug{jh}")
                    nc.gpsimd.dma_start(utj[:, :], spill[g * G + jh * 128: g * G + jh * 128 + 128, E2:H])
                    vt.append(vtj)
                    ut.append(utj)
                vt8 = p4.tile([128, 2, 2, 512], FP8, tag="vt8")
                ut8 = p4.tile([128, 2, 2, 512], FP8, tag="ut8")
                for jh in range(2):
                    for e in range(2):
                        nc.vector.tensor_copy(vt8[:, e, jh, :], vt[jh][:, e * 512:(e + 1) * 512])
                        nc.scalar.activation(ut8[:, e, jh, :], ut[jh][:, e * 512:(e + 1) * 512],
                                             AF.Copy)
                for it in range(2):
                    idx = g * 2 + it
                    islice = slice(g * G + it * 128, g * G + it * 128 + 128)
                    avp = p4p.tile([128, E2], F32, tag="avps")
                    aup = p4p.tile([128, E2], F32, tag="aups")
                    for dst, m8, lin in ((avp, vt8, linkv_sb), (aup, ut8, linku_sb)):
                        for e in range(2):
                            nc.tensor.matmul(dst[:, e * 512:(e + 1) * 512],
                                             attnT8[:, g, :, it * 128:it * 128 + 128],
                                             m8[:, e, :, :],
                                             start=True, stop=False, perf_mode=DR)
                            nc.tensor.matmul(dst[:, e * 512:(e + 1) * 512],
                                             lq_sb[:, islice], lin[:, e * 512:(e + 1) * 512],
                                             start=False, stop=True)
                    t1 = p4s.tile([128, E2], BF16, tag="t1")
                    nc.vector.scalar_tensor_tensor(out=t1[:, :], in0=avp[:, :],
                                                   scalar=1.0 / ASCALE, in1=ut[it][:, :],
                                                   op0=OP.mult, op1=OP.mult)
                    sg = p4s.tile([128, E2], BF16, tag="sg")
                    nc.scalar.activation(sg[:, :], t1[:, :], AF.Sigmoid)
                    t2 = p4s.tile([128, E2], BF16, tag="t2")
                    nc.vector.scalar_tensor_tensor(out=t2[:, :], in0=aup[:, :],
                                                   scalar=1.0 / ASCALE, in1=vt[it][:, :],
                                                   op0=OP.mult, op1=OP.mult)
                    go = p4g.tile([128, E2], BF16, tag="go")
                    nc.vector.scalar_tensor_tensor(out=go[:, :], in0=t2[:, :], scalar=1.0,
                                                   in1=sg[:, :], op0=OP.mult, op1=OP.mult,
                                                   accum_out=sums[:, idx:idx + 1])
                    jnk = p4s.tile([128, E2], BF16, tag="jnk")
                    nc.scalar.activation(jnk[:, :], go[:, :], AF.Square,
                                         accum_out=sumsq[:, idx:idx + 1])
                    govu.append(go)
                    if len(govu) == GRP:
                        _p4_norm(nc, tc, p4s, govu, sums, sumsq, eps_sb, zoT8,
                                 idx - GRP + 1)
                        govu = []

        # ---------------- P5: output FFConvM ----------------
        with tc.tile_pool(name="p5", bufs=2) as p5, \
             tc.tile_pool(name="p5d", bufs=2) as p5d, \
             tc.tile_pool(name="p5v", bufs=2) as p5v, \
             tc.tile_pool(name="p5p", bufs=2, space="PSUM") as p5p, \
             tc.tile_pool(name="p5cp", bufs=3, space="PSUM") as p5cp:
            def produce5(oc):
                dg = p5d.tile([128, 7, 2, 128], FP8, tag="dg5")
                nc.sync.dma_start(dg[:, :, :, :], diag.ap()[:, 16 + oc, :, :, :])
                hb = p5.tile([128, NPADBUF], BF16, tag="hpad5")
                nc.vector.memset(hb[:, 0:PAD], 0.0)
                nc.vector.memset(hb[:, PAD + N:], 0.0)
                for c in range(8):
                    ps = p5p.tile([128, 512], F32, tag="ops")
                    for pr in range(4):
                        nc.tensor.matmul(ps[:, :], wo_sb[:, pr, :, oc * 128:(oc + 1) * 128],
                                         zoT8[:, 2 * pr:2 * pr + 2, c * 512:(c + 1) * 512],
                                         start=(pr == 0), stop=(pr == 3), perf_mode=DR)
                    nc.scalar.activation(hb[:, PAD + c * 512:PAD + (c + 1) * 512], ps[:, :],
                                         AF.Silu, bias=bo_sb[:, oc:oc + 1], scale=1.0)
                h8 = p5.tile([128, NPADBUF], FP8, tag="h85")
                nc.scalar.activation(h8[:, :], hb[:, :], AF.Copy)
                return dg, hb, h8

            def convpost5(oc, dg, hb, h8):
                acc = p5.tile([128, N], BF16, tag="acc5")
                _emit_conv(nc, p5cp, p5v, dg, dwv_sb[:, 16 + oc, :], h8, hb, acc)
                nc.sync.dma_start_transpose(vo_big[:, :, oc, :], acc[:, :])

            prev = None
            for oc in range(4):
                pr = produce5(oc)
                if prev is not None:
                    convpost5(*prev)
                prev = (oc, *pr)
            convpost5(*prev)

        # ---------------- P6: residual (4 token-tiles per iteration) ----------------
        out_v = out.ap().rearrange("(c p) d -> p c d", p=128)
        with tc.tile_pool(name="p6", bufs=3) as p6:
            for cch in range(8):
                xt = p6.tile([128, 4, D], F32, tag="xt6")
                nc.gpsimd.dma_start(xt[:, :, :], xs_v[:, 4 * cch:4 * cch + 4, :])
                of = p6.tile([128, 4, D], F32, tag="of")
                nc.vector.tensor_tensor(
                    out=of[:, :, :].rearrange("p a c -> p (a c)"),
                    in0=xt[:, :, :].rearrange("p a c -> p (a c)"),
                    in1=vo_big[:, 4 * cch:4 * cch + 4, :, :].rearrange("p a b c -> p (a b c)"),
                    op=OP.add)
                nc.gpsimd.dma_start(out_v[:, 4 * cch:4 * cch + 4, :], of[:, :, :])
    es.close()


def _p4_norm(nc, tc, pool, gos, sums, sumsq, eps_sb, zoT8, idx0):
    """Deferred LayerNorm for GRP gating tiles: batched stats then per-tile
    normalize + transpose + fp8 convert."""
    n = len(gos)
    sl = slice(idx0, idx0 + n)
    mean = pool.tile([128, n], F32, tag="mean")
    nc.vector.tensor_scalar(out=mean[:, :], in0=sums[:, sl], scalar1=1.0 / E2,
                            scalar2=None, op0=OP.mult)
    msq = pool.tile([128, n], F32, tag="msq")
    nc.vector.tensor_tensor(out=msq[:, :], in0=mean[:, :], in1=mean[:, :], op=OP.mult)
    var = pool.tile([128, n], F32, tag="var")
    nc.vector.scalar_tensor_tensor(out=var[:, :], in0=sumsq[:, sl], scalar=1.0 / E2,
                                   in1=msq[:, :], op0=OP.mult, op1=OP.subtract)
    rstd = pool.tile([128, n], F32, tag="rstdn")
    nc.scalar.activation(rstd[:, :], var[:, :], AF.Sqrt, bias=eps_sb[:, :], scale=1.0)
    nc.vector.reciprocal(rstd[:, :], rstd[:, :])
    nmu = pool.tile([128, n], F32, tag="nmun")
    nc.vector.tensor_tensor(out=nmu[:, :], in0=mean[:, :], in1=rstd[:, :], op=OP.mult)
    nc.vector.tensor_scalar(out=nmu[:, :], in0=nmu[:, :], scalar1=-1.0,
                            scalar2=None, op0=OP.mult)
    for j, go in enumerate(gos):
        tti = idx0 + j
        zo = pool.tile([128, E2], BF16, tag="zon")
        nc.scalar.activation(zo[:, :], go[:, :], AF.Identity,
                             bias=nmu[:, j:j + 1], scale=rstd[:, j:j + 1])
        zot = pool.tile([128, 8, 128], BF16, tag="zot")
        nc.sync.dma_start_transpose(zot[:, :, :], zo[:, :])
        nc.scalar.activation(zoT8[:, :, tti * 128:(tti + 1) * 128], zot[:, :, :], AF.Copy)


def _build_nc():
    nc = bacc.Bacc("TRN2", target_bir_lowering=False, debug=False)
    x = nc.dram_tensor("x", [N, D], F32, kind="ExternalInput")
    wh8 = nc.dram_tensor("wh8", [128, 2, 2, H], FP8, kind="ExternalInput")
    wqk8 = nc.dram_tensor("wqk8", [128, 2, 2, QK], FP8, kind="ExternalInput")
    wo8 = nc.dram_tensor("wo8", [128, 4, 2, D], FP8, kind="ExternalInput")
    bh = nc.dram_tensor("bh", [128, 16], F32, kind="ExternalInput")
    bqk = nc.dram_tensor("bqk", [128, 1], F32, kind="ExternalInput")
    bo = nc.dram_tensor("bo", [128, 4], F32, kind="ExternalInput")
    gb = nc.dram_tensor("gb", [128, 8], F32, kind="ExternalInput")
    diag = nc.dram_tensor("diag", [128, NCH, 7, 2, 128], FP8, kind="ExternalInput")
    dwv = nc.dram_tensor("dwv", [128, NCH, 4], F32, kind="ExternalInput")
    out = nc.dram_tensor("out", [N, D], F32, kind="ExternalOutput")
    spill = nc.dram_tensor("spill", [N, H], BF16)
    with tile.TileContext(nc) as tc:
        _emit(nc, tc, x, wh8, wqk8, wo8, bh, bqk, bo, gb, diag, dwv, out, spill)
    nc.compile()
    return nc


def prep_inputs(inputs):
    f32 = np.float32
    fp8 = ml_dtypes.float8_e4m3
    W_h = np.asarray(inputs["W_h"], f32)
    W_qk = np.asarray(inputs["W_qk"], f32)
    W_o = np.asarray(inputs["W_o"], f32)
    whp = np.asarray(inputs["ln_h_g"], f32)[:, None] * W_h
    bhp = np.asarray(inputs["ln_h_b"], f32) @ W_h + np.asarray(inputs["b_h"], f32)
    wqkp = np.asarray(inputs["ln_qk_g"], f32)[:, None] * W_qk
    bqkp = np.asarray(inputs["ln_qk_b"], f32) @ W_qk + np.asarray(inputs["b_qk"], f32)
    wop = np.asarray(inputs["ln_o_g"], f32)[:, None] * W_o
    bop = np.asarray(inputs["ln_o_b"], f32) @ W_o + np.asarray(inputs["b_o"], f32)
    gamma = np.asarray(inputs["gamma"], f32).copy()
    beta = np.asarray(inputs["beta"], f32).copy()
    gamma[0] /= G
    beta[0] /= G
    gamma[1] *= ASCALE
    beta[1] *= ASCALE
    gamma[3] /= N
    beta[3] /= N

    def lhsT8(w, kt):
        # [din, dout] -> [128, kt/2 pairs, 2, dout] fp8
        t = w.reshape(kt, 128, -1).transpose(1, 0, 2)  # [128, kt, dout]
        return np.ascontiguousarray(
            t.reshape(128, kt // 2, 2, t.shape[-1])).astype(fp8)

    def chan(v, ntiles):
        return np.ascontiguousarray(v.reshape(ntiles, 128).T).astype(f32)

    # diagonal conv stationaries: [128, NCH, 9, 2, 128] fp8
    dw_h = np.asarray(inputs["dw_h"], f32)
    dw_o = np.asarray(inputs["dw_o"], f32)
    dw_qk = np.asarray(inputs["dw_qk"], f32)
    diag = np.zeros((128, NCH, 7, 2, 128), f32)
    dwv = np.zeros((128, NCH, 4), f32)
    rng128 = np.arange(128)
    for ct in range(NCH):
        if ct < 16:
            wsrc = dw_h[:, ct * 128:(ct + 1) * 128]
        elif ct < 20:
            wsrc = dw_o[:, (ct - 16) * 128:(ct - 15) * 128]
        else:
            wsrc = dw_qk
        for pr, (k0, k1) in enumerate(PAIRS):
            diag[rng128, ct, pr, 0, rng128] = wsrc[k0]
            diag[rng128, ct, pr, 1, rng128] = wsrc[k1]
        diag[rng128, ct, 6, 0, rng128] = wsrc[16]
        for j, k in enumerate(DVETAPS):
            dwv[:, ct, j] = wsrc[k]
    return {
        "wh8": lhsT8(whp, 4), "wqk8": lhsT8(wqkp, 4), "wo8": lhsT8(wop, 8),
        "bh": chan(bhp, 16), "bqk": chan(bqkp, 1), "bo": chan(bop, 4),
        "gb": np.concatenate([gamma.T, beta.T], axis=1).astype(f32),
        "diag": diag.astype(fp8), "dwv": dwv,
    }


_NC = None


def get_nc():
    global _NC
    if _NC is None:
        _NC = _build_nc()
    return _NC


def make_in_maps(inputs):
    x = np.asarray(inputs["x"], np.float32)
    B = x.shape[0]
    prep = prep_inputs(inputs)
    return [{"x": np.ascontiguousarray(x[b]), **prep} for b in range(B)]


def kernel(**inputs):
    nc = get_nc()
    in_maps = make_in_maps(inputs)
    res = bass_utils.run_bass_kernel_spmd(nc, in_maps, core_ids=list(range(8)))
    out = np.stack([res.results[b]["out"] for b in range(8)], axis=0)
    return out.astype(np.float32)


# revision 21
# speedup vs baseline: 1.6599x; 1.0298x over previous
"""Self-contained TRN2 kernel for nn_FLASH_ShareA_FFConvM_FlashAttn.

kernel(**inputs) takes the full (unsharded) inputs from setup_inputs() and
returns the full (B, N, D) float32 output. Internally: data-parallel over the
batch — one batch sample per NeuronCore, 8 cores, no collectives.

v2: all heavy matmuls in fp8 DoubleRow (paired k-tiles / paired conv taps),
depthwise convs fully on the PE as paired diagonal matmuls, attention weights
pre-scaled by 2^30 to stay in fp8 range, deferred output LayerNorm, and the
zspill round-trip replaced by an SBUF-resident transposed buffer.
"""
import sys

if "/opt/trn_rl_repo" not in sys.path:
    sys.path.insert(0, "/opt/trn_rl_repo")

import numpy as np
import ml_dtypes
import concourse.bass as bass
import concourse.bacc as bacc
import concourse.mybir as mybir
import concourse.tile as tile
from concourse import bass_utils
from concourse.ap import AP

F32 = mybir.dt.float32
BF16 = mybir.dt.bfloat16
FP8 = mybir.dt.float8e4
AF = mybir.ActivationFunctionType
OP = mybir.AluOpType
DR = mybir.MatmulPerfMode.DoubleRow

N, D, H, QK, G = 4096, 512, 2048, 128, 256
NG = N // G
NT = N // 128
KTAPS = 17
PAD = 8
NPADBUF = N + 2 * PAD  # fp8/bf16 padded conv input length (max tap read = N+15)
E2 = 2 * D
EPS = 1e-5
NCH = 21  # conv channel tiles: hid 0..15, out 16..19, qk 20
# conv tap pairs with stride-4 spacing (DR rows must be >=4 fp8 elements apart).
# Taps 8,12,9,13 run on the DVE instead (frees PE passes during P3/P5).
PAIRS = [(0, 4), (1, 5), (2, 6), (3, 7), (10, 14), (11, 15)]
DVETAPS = [8, 12, 9, 13]
ASCALE = float(2 ** 30)      # attention-weight scale kept inside psum
RELUSC = float(2 ** 15)      # sqrt(ASCALE), applied before squaring
GRP = 16                     # P4 deferred-LN batch size (iterations)


def _pair_ap(t, off, n):
    """Overlapping [128, 2, n] moving AP: row j reads t[:, off+4j : off+4j+n]."""
    base = t[:, 0:1]
    return AP(base.tensor, base.offset + off, [list(base.ap[0]), [4, 2], [1, n]])


def _emit_conv(nc, pool, dvp, dg, wv, h8t, hb, acc):
    """acc = h + conv(h): 6 DR tap pairs + tap16 on PE; 4 taps + identity on DVE."""
    # DVE partial: accd = h + sum_{k in DVETAPS} w_k * h_shift_k   (full width)
    accd = dvp.tile([128, N], BF16, tag="accd")
    tmp = dvp.tile([128, N], BF16, tag="dvtmp")
    for j, k in enumerate(DVETAPS):
        nc.vector.tensor_scalar(out=tmp[:, :], in0=hb[:, k:k + N],
                                scalar1=wv[:, j:j + 1], scalar2=None, op0=OP.mult)
        if j == 0:
            nc.vector.tensor_tensor(out=accd[:, :], in0=tmp[:, :],
                                    in1=hb[:, PAD:PAD + N], op=OP.add)
        else:
            nc.vector.tensor_tensor(out=accd[:, :], in0=tmp[:, :],
                                    in1=accd[:, :], op=OP.add)
    for c in range(8):
        cb = c * 512
        cp = pool.tile([128, 512], F32, tag="convps")
        for pr in range(6):
            nc.tensor.matmul(cp[:, :], dg[:, pr, :, :], _pair_ap(h8t, PAIRS[pr][0] + cb, 512),
                             start=(pr == 0), stop=False, perf_mode=DR)
        nc.tensor.matmul(cp[:, :], dg[:, 6, 0, :], h8t[:, 16 + cb:16 + cb + 512],
                         start=False, stop=True)
        nc.vector.tensor_tensor(out=acc[:, cb:cb + 512], in0=cp[:, :],
                                in1=accd[:, cb:cb + 512], op=OP.add)


def _emit(nc, tc, x, wh8, wqk8, wo8, bh, bqk, bo, gb, diag, dwv, out, spill):
    from contextlib import ExitStack
    es = ExitStack()
    consts = es.enter_context(tc.tile_pool(name="consts", bufs=1))
    wh_sb = consts.tile([128, 2, 2, H], FP8)
    nc.sync.dma_start(wh_sb[:, :, :, :], wh8.ap())
    wqk_sb = consts.tile([128, 2, 2, QK], FP8)
    nc.sync.dma_start(wqk_sb[:, :, :, :], wqk8.ap())
    wo_sb = consts.tile([128, 4, 2, D], FP8)
    nc.sync.dma_start(wo_sb[:, :, :, :], wo8.ap())
    bh_sb = consts.tile([128, 16], F32)
    nc.sync.dma_start(bh_sb[:, :], bh.ap())
    bqk_sb = consts.tile([128, 1], F32)
    nc.sync.dma_start(bqk_sb[:, :], bqk.ap())
    bo_sb = consts.tile([128, 4], F32)
    nc.sync.dma_start(bo_sb[:, :], bo.ap())
    gb_sb = consts.tile([128, 8], F32)
    nc.sync.dma_start(gb_sb[:, :], gb.ap())
    dwv_sb = consts.tile([128, NCH, 4], F32)
    nc.sync.dma_start(dwv_sb[:, :, :], dwv.ap())
    eps_sb = consts.tile([128, 1], F32)
    nc.vector.memset(eps_sb[:, :], EPS)

    outer = es.enter_context(tc.tile_pool(name="outer", bufs=1))
    attnT8 = outer.tile([128, NG, 2, G], FP8)
    lq_sb = outer.tile([128, N], BF16)
    lk_str = outer.tile([128, NT, 128], BF16)
    linkv_sb = outer.tile([128, E2], BF16)
    linku_sb = outer.tile([128, E2], BF16)
    sums = outer.tile([128, 32], F32)
    sumsq = outer.tile([128, 32], F32)
    qq_sb = outer.tile([128, N], BF16)
    qkk_sb = outer.tile([128, N], BF16)
    spill_v = spill.ap().rearrange("(tt p) (q c4) -> p tt q c4", p=128, c4=512)

    es2 = ExitStack()
    zpool = es2.enter_context(tc.tile_pool(name="zpool", bufs=1))
    zT8 = []
    for c in range(8):
        zT8c = zpool.tile([128, 4, 512], FP8, tag=f"zT8_{c}")
        zT8.append(zT8c)

    # ---------------- P0: token-shifted LayerNorm -> zT8 chunks ----------------
    # x loads batched 4 token-tiles per DMA; shifted first-half channels loaded
    # separately with a one-row offset.
    xs_v = x.ap().rearrange("(c p) d -> p c d", p=128)
    with tc.tile_pool(name="p0", bufs=3) as p0, \
         tc.tile_pool(name="p0z", bufs=3) as p0z, \
         tc.tile_pool(name="p0s", bufs=6) as p0s:
        for cch in range(8):
            x4 = p0.tile([128, 4, D], F32, tag="x4")
            t0 = cch * 512
            if cch == 0:
                nc.vector.memset(x4[0:1, 0, 0:D // 2], 0.0)
                nc.gpsimd.dma_start(x4[1:128, 0, 0:D // 2], x[0:127, 0:D // 2])
                for j in range(1, 4):
                    nc.gpsimd.dma_start(x4[:, j, 0:D // 2],
                                        x[t0 + j * 128 - 1:t0 + j * 128 + 127, 0:D // 2])
            else:
                nc.gpsimd.dma_start(
                    x4[:, :, 0:D // 2],
                    x.ap()[t0 - 1:t0 + 511, 0:D // 2].rearrange("(j p) d -> p j d", p=128))
            nc.gpsimd.dma_start(x4[:, :, D // 2:D],
                                xs_v[:, 4 * cch:4 * cch + 4, D // 2:D])
            ztc = p0z.tile([128, 4, 512], BF16, tag="ztc")
            for j in range(4):
                st6 = p0s.tile([128, 6], F32, tag="st6")
                nc.vector.bn_stats(st6[:, :], x4[:, j, :])
                mv = p0s.tile([128, 2], F32, tag="mv")
                nc.vector.bn_aggr(mv[:, :], st6[:, :])
                rstd = p0s.tile([128, 1], F32, tag="rstd")
                nc.scalar.activation(rstd[:, :], mv[:, 1:2], AF.Sqrt, bias=eps_sb[:, :], scale=1.0)
                nc.vector.reciprocal(rstd[:, :], rstd[:, :])
                nmu = p0s.tile([128, 1], F32, tag="nmu")
                nc.vector.tensor_scalar(out=nmu[:, :], in0=mv[:, 0:1], scalar1=rstd[:, :],
                                        scalar2=-1.0, op0=OP.mult, op1=OP.mult)
                zt = p0.tile([128, D], BF16, tag="zt")
                nc.scalar.activation(zt[:, :], x4[:, j, :], AF.Identity,
                                     bias=nmu[:, :], scale=rstd[:, :])
                eng = nc.sync if j % 2 == 0 else nc.scalar
                eng.dma_start_transpose(ztc[:, :, j * 128:j * 128 + 128], zt[:, :])
                nc.scalar.activation(zT8[cch][:, :, j * 128:j * 128 + 128],
                                     ztc[:, :, j * 128:j * 128 + 128], AF.Copy)

    # ---------------- P1+P3: qk path interleaved with hidden FFConvM ----------------
    with tc.tile_pool(name="p1", bufs=1) as p1, \
         tc.tile_pool(name="p3", bufs=2) as p3, \
         tc.tile_pool(name="p3d", bufs=3) as p3d, \
         tc.tile_pool(name="p3q", bufs=1) as p3q, \
         tc.tile_pool(name="p3v", bufs=2) as p3v, \
         tc.tile_pool(name="p1s", bufs=3) as p1s, \
         tc.tile_pool(name="p3p", bufs=3, space="PSUM") as p3p, \
         tc.tile_pool(name="p3cp", bufs=3, space="PSUM") as p3cp, \
         tc.tile_pool(name="p1sp", bufs=1, space="PSUM") as p1sp, \
         tc.tile_pool(name="p3lin", bufs=1, space="PSUM") as p3lin:
        state = {"strips4": None}

        def produce_start(hc):
            dg = p3d.tile([128, 7, 2, 128], FP8, tag="dg")
            nc.gpsimd.dma_start(dg[:, :, :, :], diag.ap()[:, hc, :, :, :])
            hb = p3.tile([128, NPADBUF], BF16, tag="hpad")
            nc.vector.memset(hb[:, 0:PAD], 0.0)
            nc.vector.memset(hb[:, PAD + N:], 0.0)
            return dg, hb

        def produce_chunk(hc, hb, c):
            ps = p3p.tile([128, 512], F32, tag="hps")
            for pr in range(2):
                nc.tensor.matmul(ps[:, :], wh_sb[:, pr, :, hc * 128:(hc + 1) * 128],
                                 zT8[c][:, 2 * pr:2 * pr + 2, :],
                                 start=(pr == 0), stop=(pr == 1), perf_mode=DR)
            nc.scalar.activation(hb[:, PAD + c * 512:PAD + (c + 1) * 512], ps[:, :],
                                 AF.Silu, bias=bh_sb[:, hc:hc + 1], scale=1.0)

        def produce_end(hb):
            h8 = p3.tile([128, NPADBUF], FP8, tag="h8pad")
            nc.scalar.activation(h8[:, :], hb[:, :], AF.Copy)
            return h8

        def produce(hc):
            dg, hb = produce_start(hc)
            for c in range(8):
                produce_chunk(hc, hb, c)
            return dg, hb, produce_end(hb)

        def convpost(hc, dg, hb, h8):
            if hc % 4 == 0:
                s4_new = p3q.tile([128, NT, 4, 128], BF16, tag="strips4")
                state["strips4"] = s4_new
            strips4 = state["strips4"]
            acc = p3.tile([128, N], BF16, tag="acc")
            _emit_conv(nc, p3cp, p3v, dg, dwv_sb[:, hc, :], h8, hb, acc)
            nc.sync.dma_start_transpose(strips4[:, :, hc % 4, :], acc[:, :])
            if hc % 4 == 3:
                q = hc // 4
                nc.gpsimd.dma_start(spill_v[:, :, q, :], strips4[:, :, :, :])
                kvp = p3lin.tile([128, 512], F32, tag="kvps")
                for tt in range(NT):
                    nc.tensor.matmul(
                        kvp[:, :], lk_str[:, tt, :],
                        strips4[:, tt, :, :].rearrange("p a c -> p (a c)"),
                        start=(tt == 0), stop=(tt == NT - 1))
                dst = linkv_sb if q < 2 else linku_sb
                nc.scalar.activation(dst[:, (q % 2) * 512:(q % 2) * 512 + 512],
                                     kvp[:, :], AF.Copy)

        # Front: chunk-major over {qk, hc0, hc1} so the PE consumes zT8 chunks
        # the moment P0 produces them (no head-of-line blocking on later chunks).
        dgq = p3d.tile([128, 7, 2, 128], FP8, tag="dg")
        nc.gpsimd.dma_start(dgq[:, :, :, :], diag.ap()[:, 20, :, :, :])
        qkp = p1.tile([128, NPADBUF], BF16, tag="qkpad")
        nc.vector.memset(qkp[:, 0:PAD], 0.0)
        nc.vector.memset(qkp[:, PAD + N:], 0.0)
        q8p = p1.tile([128, NPADBUF], FP8, tag="qk8pad")
        dg0, hb0 = produce_start(0)
        dg1, hb1 = produce_start(1)
        for c in range(8):
            ps = p3p.tile([128, 512], F32, tag="hps")
            for pr in range(2):
                nc.tensor.matmul(ps[:, :], wqk_sb[:, pr, :, :],
                                 zT8[c][:, 2 * pr:2 * pr + 2, :],
                                 start=(pr == 0), stop=(pr == 1), perf_mode=DR)
            nc.scalar.activation(qkp[:, PAD + c * 512:PAD + (c + 1) * 512], ps[:, :],
                                 AF.Silu, bias=bqk_sb[:, :], scale=1.0)
            produce_chunk(0, hb0, c)
            produce_chunk(1, hb1, c)
        nc.scalar.activation(q8p[:, :], qkp[:, :], AF.Copy)
        h80 = produce_end(hb0)
        h81 = produce_end(hb1)
        qkc = p3.tile([128, N], BF16, tag="acc")
        _emit_conv(nc, p3cp, p3v, dgq, dwv_sb[:, 20, :], q8p, qkp, qkc)
        lkk = p3.tile([128, N], BF16, tag="acc")
        for i, dst in ((0, qq_sb), (1, lq_sb), (2, qkk_sb), (3, lkk)):
            nc.vector.tensor_scalar(out=dst[:, :], in0=qkc[:, :], scalar1=gb_sb[:, i:i + 1],
                                    scalar2=gb_sb[:, 4 + i:5 + i], op0=OP.mult, op1=OP.add)
        nc.sync.dma_start_transpose(lk_str[:, :, :], lkk[:, :])
        convpost(0, dg0, hb0, h80)
        for g in range(NG):
            for jh in range(2):
                sp = p1sp.tile([128, G], F32, tag="simps")
                nc.tensor.matmul(sp[:, :],
                                 qkk_sb[:, g * G + jh * 128: g * G + jh * 128 + 128],
                                 qq_sb[:, g * G:(g + 1) * G],
                                 start=True, stop=True)
                rel = p1s.tile([128, G], BF16, tag="rel")
                nc.scalar.activation(rel[:, :], sp[:, :], AF.Relu, scale=RELUSC)
                nc.scalar.activation(attnT8[:, g, jh, :], rel[:, :], AF.Square)

        prev = (1, dg1, hb1, h81)
        for hc in range(2, 16):
            pr = produce(hc)
            convpost(*prev)
            prev = (hc, *pr)
        convpost(*prev)

    es2.close()  # frees zT8 chunks before the P4/P5 pools open

    # ---------------- P4: attention apply + gating (deferred LN) ----------------
    with tc.tile_pool(name="mid", bufs=1) as mid:
        zoT8 = mid.tile([128, 8, N], FP8)
        vo_big = mid.tile([128, NT, 4, 128], BF16)
        with tc.tile_pool(name="p4", bufs=2) as p4, \
             tc.tile_pool(name="p4g", bufs=GRP + 2) as p4g, \
             tc.tile_pool(name="p4s", bufs=2) as p4s, \
             tc.tile_pool(name="p4p", bufs=2, space="PSUM") as p4p:
            govu = []   # (go, vt?, ...) per pending it in current group
            for g in range(NG):
                vt, ut = [], []
                for jh in range(2):
                    vtj = p4.tile([128, E2], BF16, tag=f"vg{jh}")
                    nc.gpsimd.dma_start(vtj[:, :], spill[g * G + jh * 128: g * G + jh * 128 + 128, 0:E2])
                    utj = p4.tile([128, E2], BF16, tag=f"ug{jh}")
                    nc.gpsimd.dma_start(utj[:, :], spill[g * G + jh * 128: g * G + jh * 128 + 128, E2:H])
                    vt.append(vtj)
                    ut.append(utj)
                vt8 = p4.tile([128, 2, 2, 512], FP8, tag="vt8")
                ut8 = p4.tile([128, 2, 2, 512], FP8, tag="ut8")
                for jh in range(2):
                    for e in range(2):
                        nc.vector.tensor_copy(vt8[:, e, jh, :], vt[jh][:, e * 512:(e + 1) * 512])
                        nc.scalar.activation(ut8[:, e, jh, :], ut[jh][:, e * 512:(e + 1) * 512],
                                             AF.Copy)
                for it in range(2):
                    idx = g * 2 + it
                    islice = slice(g * G + it * 128, g * G + it * 128 + 128)
                    avp = p4p.tile([128, E2], F32, tag="avps")
                    aup = p4p.tile([128, E2], F32, tag="aups")
                    for dst, m8, lin in ((avp, vt8, linkv_sb), (aup, ut8, linku_sb)):
                        for e in range(2):
                            nc.tensor.matmul(dst[:, e * 512:(e + 1) * 512],
                                             attnT8[:, g, :, it * 128:it * 128 + 128],
                                             m8[:, e, :, :],
                                             start=True, stop=False, perf_mode=DR)
                            nc.tensor.matmul(dst[:, e * 512:(e + 1) * 512],
                                             lq_sb[:, islice], lin[:, e * 512:(e + 1) * 512],
                                             start=False, stop=True)
                    t1 = p4s.tile([128, E2], BF16, tag="t1")
                    nc.vector.scalar_tensor_tensor(out=t1[:, :], in0=avp[:, :],
                                                   scalar=1.0 / ASCALE, in1=ut[it][:, :],
                                                   op0=OP.mult, op1=OP.mult)
                    sg = p4s.tile([128, E2], BF16, tag="sg")
                    nc.scalar.activation(sg[:, :], t1[:, :], AF.Sigmoid)
                    t2 = p4s.tile([128, E2], BF16, tag="t2")
                    nc.vector.scalar_tensor_tensor(out=t2[:, :], in0=aup[:, :],
                                                   scalar=1.0 / ASCALE, in1=vt[it][:, :],
                                                   op0=OP.mult, op1=OP.mult)
                    go = p4g.tile([128, E2], BF16, tag="go")
                    nc.vector.scalar_tensor_tensor(out=go[:, :], in0=t2[:, :], scalar=1.0,
                                                   in1=sg[:, :], op0=OP.mult, op1=OP.mult,
                                                   accum_out=sums[:, idx:idx + 1])
                    jnk = p4s.tile([128, E2], BF16, tag="jnk")
                    nc.scalar.activation(jnk[:, :], go[:, :], AF.Square,
                                         accum_out=sumsq[:, idx:idx + 1])
                    govu.append(go)
                    if len(govu) == GRP:
                        _p4_norm(nc, tc, p4s, govu, sums, sumsq, eps_sb, zoT8,
                                 idx - GRP + 1)
                        govu = []

        # ---------------- P5: output FFConvM ----------------
        with tc.tile_pool(name="p5", bufs=2) as p5, \
             tc.tile_pool(name="p5d", bufs=2) as p5d, \
             tc.tile_pool(name="p5v", bufs=2) as p5v, \
             tc.tile_pool(name="p5p", bufs=2, space="PSUM") as p5p, \
             tc.tile_pool(name="p5cp", bufs=3, space="PSUM") as p5cp:
            def produce5(oc):
                dg = p5d.tile([128, 7, 2, 128], FP8, tag="dg5")
                nc.gpsimd.dma_start(dg[:, :, :, :], diag.ap()[:, 16 + oc, :, :, :])
                hb = p5.tile([128, NPADBUF], BF16, tag="hpad5")
                nc.vector.memset(hb[:, 0:PAD], 0.0)
                nc.vector.memset(hb[:, PAD + N:], 0.0)
                for c in range(8):
                    ps = p5p.tile([128, 512], F32, tag="ops")
                    for pr in range(4):
                        nc.tensor.matmul(ps[:, :], wo_sb[:, pr, :, oc * 128:(oc + 1) * 128],
                                         zoT8[:, 2 * pr:2 * pr + 2, c * 512:(c + 1) * 512],
                                         start=(pr == 0), stop=(pr == 3), perf_mode=DR)
                    nc.scalar.activation(hb[:, PAD + c * 512:PAD + (c + 1) * 512], ps[:, :],
                                         AF.Silu, bias=bo_sb[:, oc:oc + 1], scale=1.0)
                h8 = p5.tile([128, NPADBUF], FP8, tag="h85")
                nc.scalar.activation(h8[:, :], hb[:, :], AF.Copy)
                return dg, hb, h8

            def convpost5(oc, dg, hb, h8):
                acc = p5.tile([128, N], BF16, tag="acc5")
                _emit_conv(nc, p5cp, p5v, dg, dwv_sb[:, 16 + oc, :], h8, hb, acc)
                nc.sync.dma_start_transpose(vo_big[:, :, oc, :], acc[:, :])

            prev = None
            for oc in range(4):
                pr = produce5(oc)
                if prev is not None:
                    convpost5(*prev)
                prev = (oc, *pr)
            convpost5(*prev)

        # ---------------- P6: residual (4 token-tiles per iteration) ----------------
        out_v = out.ap().rearrange("(c p) d -> p c d", p=128)
        with tc.tile_pool(name="p6", bufs=3) as p6:
            for cch in range(8):
                xt = p6.tile([128, 4, D], F32, tag="xt6")
                nc.gpsimd.dma_start(xt[:, :, :], xs_v[:, 4 * cch:4 * cch + 4, :])
                of = p6.tile([128, 4, D], F32, tag="of")
                nc.vector.tensor_tensor(
                    out=of[:, :, :].rearrange("p a c -> p (a c)"),
                    in0=xt[:, :, :].rearrange("p a c -> p (a c)"),
                    in1=vo_big[:, 4 * cch:4 * cch + 4, :, :].rearrange("p a b c -> p (a b c)"),
                    op=OP.add)
                nc.gpsimd.dma_start(out_v[:, 4 * cch:4 * cch + 4, :], of[:, :, :])
    es.close()


def _p4_norm(nc, tc, pool, gos, sums, sumsq, eps_sb, zoT8, idx0):
    """Deferred LayerNorm for GRP gating tiles: batched stats then per-tile
    normalize + transpose + fp8 convert."""
    n = len(gos)
    sl = slice(idx0, idx0 + n)
    mean = pool.tile([128, n], F32, tag="mean")
    nc.vector.tensor_scalar(out=mean[:, :], in0=sums[:, sl], scalar1=1.0 / E2,
                            scalar2=None, op0=OP.mult)
    msq = pool.tile([128, n], F32, tag="msq")
    nc.vector.tensor_tensor(out=msq[:, :], in0=mean[:, :], in1=mean[:, :], op=OP.mult)
    var = pool.tile([128, n], F32, tag="var")
    nc.vector.scalar_tensor_tensor(out=var[:, :], in0=sumsq[:, sl], scalar=1.0 / E2,
                                   in1=msq[:, :], op0=OP.mult, op1=OP.subtract)
    rstd = pool.tile([128, n], F32, tag="rstdn")
    nc.scalar.activation(rstd[:, :], var[:, :], AF.Sqrt, bias=eps_sb[:, :], scale=1.0)
    nc.vector.reciprocal(rstd[:, :], rstd[:, :])
    nmu = pool.tile([128, n], F32, tag="nmun")
    nc.vector.tensor_tensor(out=nmu[:, :], in0=mean[:, :], in1=rstd[:, :], op=OP.mult)
    nc.vector.tensor_scalar(out=nmu[:, :], in0=nmu[:, :], scalar1=-1.0,
                            scalar2=None, op0=OP.mult)
    for j, go in enumerate(gos):
        tti = idx0 + j
        zo = pool.tile([128, E2], BF16, tag="zon")
        nc.scalar.activation(zo[:, :], go[:, :], AF.Identity,
                             bias=nmu[:, j:j + 1], scale=rstd[:, j:j + 1])
        zot = pool.tile([128, 8, 128], BF16, tag="zot")
        nc.sync.dma_start_transpose(zot[:, :, :], zo[:, :])
        nc.scalar.activation(zoT8[:, :, tti * 128:(tti + 1) * 128], zot[:, :, :], AF.Copy)


def _build_nc():
    nc = bacc.Bacc("TRN2", target_bir_lowering=False, debug=False)
    x = nc.dram_tensor("x", [N, D], F32, kind="ExternalInput")
    wh8 = nc.dram_tensor("wh8", [128, 2, 2, H], FP8, kind="ExternalInput")
    wqk8 = nc.dram_tensor("wqk8", [128, 2, 2, QK], FP8, kind="ExternalInput")
    wo8 = nc.dram_tensor("wo8", [128, 4, 2, D], FP8, kind="ExternalInput")
    bh = nc.dram_tensor("bh", [128, 16], F32, kind="ExternalInput")
    bqk = nc.dram_tensor("bqk", [128, 1], F32, kind="ExternalInput")
    bo = nc.dram_tensor("bo", [128, 4], F32, kind="ExternalInput")
    gb = nc.dram_tensor("gb", [128, 8], F32, kind="ExternalInput")
    diag = nc.dram_tensor("diag", [128, NCH, 7, 2, 128], FP8, kind="ExternalInput")
    dwv = nc.dram_tensor("dwv", [128, NCH, 4], F32, kind="ExternalInput")
    out = nc.dram_tensor("out", [N, D], F32, kind="ExternalOutput")
    spill = nc.dram_tensor("spill", [N, H], BF16)
    with tile.TileContext(nc) as tc:
        _emit(nc, tc, x, wh8, wqk8, wo8, bh, bqk, bo, gb, diag, dwv, out, spill)
    nc.compile()
    return nc


def prep_inputs(inputs):
    f32 = np.float32
    fp8 = ml_dtypes.float8_e4m3
    W_h = np.asarray(inputs["W_h"], f32)
    W_qk = np.asarray(inputs["W_qk"], f32)
    W_o = np.asarray(inputs["W_o"], f32)
    whp = np.asarray(inputs["ln_h_g"], f32)[:, None] * W_h
    bhp = np.asarray(inputs["ln_h_b"], f32) @ W_h + np.asarray(inputs["b_h"], f32)
    wqkp = np.asarray(inputs["ln_qk_g"], f32)[:, None] * W_qk
    bqkp = np.asarray(inputs["ln_qk_b"], f32) @ W_qk + np.asarray(inputs["b_qk"], f32)
    wop = np.asarray(inputs["ln_o_g"], f32)[:, None] * W_o
    bop = np.asarray(inputs["ln_o_b"], f32) @ W_o + np.asarray(inputs["b_o"], f32)
    gamma = np.asarray(inputs["gamma"], f32).copy()
    beta = np.asarray(inputs["beta"], f32).copy()
    gamma[0] /= G
    beta[0] /= G
    gamma[1] *= ASCALE
    beta[1] *= ASCALE
    gamma[3] /= N
    beta[3] /= N

    def lhsT8(w, kt):
        # [din, dout] -> [128, kt/2 pairs, 2, dout] fp8
        t = w.reshape(kt, 128, -1).transpose(1, 0, 2)  # [128, kt, dout]
        return np.ascontiguousarray(
            t.reshape(128, kt // 2, 2, t.shape[-1])).astype(fp8)

    def chan(v, ntiles):
        return np.ascontiguousarray(v.reshape(ntiles, 128).T).astype(f32)

    # diagonal conv stationaries: [128, NCH, 9, 2, 128] fp8
    dw_h = np.asarray(inputs["dw_h"], f32)
    dw_o = np.asarray(inputs["dw_o"], f32)
    dw_qk = np.asarray(inputs["dw_qk"], f32)
    diag = np.zeros((128, NCH, 7, 2, 128), f32)
    dwv = np.zeros((128, NCH, 4), f32)
    rng128 = np.arange(128)
    for ct in range(NCH):
        if ct < 16:
            wsrc = dw_h[:, ct * 128:(ct + 1) * 128]
        elif ct < 20:
            wsrc = dw_o[:, (ct - 16) * 128:(ct - 15) * 128]
        else:
            wsrc = dw_qk
        for pr, (k0, k1) in enumerate(PAIRS):
            diag[rng128, ct, pr, 0, rng128] = wsrc[k0]
            diag[rng128, ct, pr, 1, rng128] = wsrc[k1]
        diag[rng128, ct, 6, 0, rng128] = wsrc[16]
        for j, k in enumerate(DVETAPS):
            dwv[:, ct, j] = wsrc[k]
    return {
        "wh8": lhsT8(whp, 4), "wqk8": lhsT8(wqkp, 4), "wo8": lhsT8(wop, 8),
        "bh": chan(bhp, 16), "bqk": chan(bqkp, 1), "bo": chan(bop, 4),
        "gb": np.concatenate([gamma.T, beta.T], axis=1).astype(f32),
        "diag": diag.astype(fp8), "dwv": dwv,
    }


_NC = None


def get_nc():
    global _NC
    if _NC is None:
        _NC = _build_nc()
    return _NC


def make_in_maps(inputs):
    x = np.asarray(inputs["x"], np.float32)
    B = x.shape[0]
    prep = prep_inputs(inputs)
    return [{"x": np.ascontiguousarray(x[b]), **prep} for b in range(B)]


def kernel(**inputs):
    nc = get_nc()
    in_maps = make_in_maps(inputs)
    res = bass_utils.run_bass_kernel_spmd(nc, in_maps, core_ids=list(range(8)))
    out = np.stack([res.results[b]["out"] for b in range(8)], axis=0)
    return out.astype(np.float32)


# revision 22
# speedup vs baseline: 1.7413x; 1.0490x over previous
"""Self-contained TRN2 kernel for nn_FLASH_ShareA_FFConvM_FlashAttn.

kernel(**inputs) takes the full (unsharded) inputs from setup_inputs() and
returns the full (B, N, D) float32 output. Internally: data-parallel over the
batch — one batch sample per NeuronCore, 8 cores, no collectives.

v2: all heavy matmuls in fp8 DoubleRow (paired k-tiles / paired conv taps),
depthwise convs fully on the PE as paired diagonal matmuls, attention weights
pre-scaled by 2^30 to stay in fp8 range, deferred output LayerNorm, and the
zspill round-trip replaced by an SBUF-resident transposed buffer.
"""
import sys

if "/opt/trn_rl_repo" not in sys.path:
    sys.path.insert(0, "/opt/trn_rl_repo")

import numpy as np
import ml_dtypes
import concourse.bass as bass
import concourse.bacc as bacc
import concourse.mybir as mybir
import concourse.tile as tile
from concourse import bass_utils
from concourse.ap import AP

F32 = mybir.dt.float32
BF16 = mybir.dt.bfloat16
FP8 = mybir.dt.float8e4
AF = mybir.ActivationFunctionType
OP = mybir.AluOpType
DR = mybir.MatmulPerfMode.DoubleRow

N, D, H, QK, G = 4096, 512, 2048, 128, 256
NG = N // G
NT = N // 128
KTAPS = 17
PAD = 8
NPADBUF = N + 2 * PAD  # fp8/bf16 padded conv input length (max tap read = N+15)
E2 = 2 * D
EPS = 1e-5
NCH = 21  # conv channel tiles: hid 0..15, out 16..19, qk 20
# conv tap pairs with stride-4 spacing (DR rows must be >=4 fp8 elements apart).
# Taps 8,12,9,13 run on the DVE instead (frees PE passes during P3/P5).
PAIRS = [(0, 4), (1, 5), (2, 6), (3, 7), (10, 14), (11, 15)]
DVETAPS = [8, 12, 9, 13]
ASCALE = float(2 ** 30)      # attention-weight scale kept inside psum
RELUSC = float(2 ** 15)      # sqrt(ASCALE), applied before squaring
GRP = 16                     # P4 deferred-LN batch size (iterations)


def _pair_ap(t, off, n):
    """Overlapping [128, 2, n] moving AP: row j reads t[:, off+4j : off+4j+n]."""
    base = t[:, 0:1]
    return AP(base.tensor, base.offset + off, [list(base.ap[0]), [4, 2], [1, n]])


def _emit_conv(nc, pool, dvp, dg, wv, h8t, hb, acc):
    """acc = h + conv(h): 6 DR tap pairs + tap16 on PE; 4 taps + identity on DVE."""
    # DVE partial: accd = h + sum_{k in DVETAPS} w_k * h_shift_k   (full width)
    accd = dvp.tile([128, N], BF16, tag="accd")
    tmp = dvp.tile([128, N], BF16, tag="dvtmp")
    for j, k in enumerate(DVETAPS):
        nc.vector.tensor_scalar(out=tmp[:, :], in0=hb[:, k:k + N],
                                scalar1=wv[:, j:j + 1], scalar2=None, op0=OP.mult)
        if j == 0:
            nc.vector.tensor_tensor(out=accd[:, :], in0=tmp[:, :],
                                    in1=hb[:, PAD:PAD + N], op=OP.add)
        else:
            nc.vector.tensor_tensor(out=accd[:, :], in0=tmp[:, :],
                                    in1=accd[:, :], op=OP.add)
    for c in range(8):
        cb = c * 512
        cp = pool.tile([128, 512], F32, tag="convps")
        for pr in range(6):
            nc.tensor.matmul(cp[:, :], dg[:, pr, :, :], _pair_ap(h8t, PAIRS[pr][0] + cb, 512),
                             start=(pr == 0), stop=False, perf_mode=DR)
        nc.tensor.matmul(cp[:, :], dg[:, 6, 0, :], h8t[:, 16 + cb:16 + cb + 512],
                         start=False, stop=True)
        nc.vector.tensor_tensor(out=acc[:, cb:cb + 512], in0=cp[:, :],
                                in1=accd[:, cb:cb + 512], op=OP.add)


def _emit(nc, tc, x, wh8, wqk8, wo8, bh, bqk, bo, gb, diag, dwv, out, spill):
    from contextlib import ExitStack
    es = ExitStack()
    consts = es.enter_context(tc.tile_pool(name="consts", bufs=1))
    wh_sb = consts.tile([128, 2, 2, H], FP8)
    nc.sync.dma_start(wh_sb[:, :, :, :], wh8.ap())
    wqk_sb = consts.tile([128, 2, 2, QK], FP8)
    nc.sync.dma_start(wqk_sb[:, :, :, :], wqk8.ap())
    wo_sb = consts.tile([128, 4, 2, D], FP8)
    nc.sync.dma_start(wo_sb[:, :, :, :], wo8.ap())
    bh_sb = consts.tile([128, 16], F32)
    nc.sync.dma_start(bh_sb[:, :], bh.ap())
    bqk_sb = consts.tile([128, 1], F32)
    nc.sync.dma_start(bqk_sb[:, :], bqk.ap())
    bo_sb = consts.tile([128, 4], F32)
    nc.sync.dma_start(bo_sb[:, :], bo.ap())
    gb_sb = consts.tile([128, 8], F32)
    nc.sync.dma_start(gb_sb[:, :], gb.ap())
    dwv_sb = consts.tile([128, NCH, 4], F32)
    nc.sync.dma_start(dwv_sb[:, :, :], dwv.ap())
    eps_sb = consts.tile([128, 1], F32)
    nc.vector.memset(eps_sb[:, :], EPS)

    outer = es.enter_context(tc.tile_pool(name="outer", bufs=1))
    attnT8 = outer.tile([128, NG, 2, G], FP8)
    lq_sb = outer.tile([128, N], BF16)
    lk_str = outer.tile([128, NT, 128], BF16)
    linkv_sb = outer.tile([128, E2], BF16)
    linku_sb = outer.tile([128, E2], BF16)
    sums = outer.tile([128, 32], F32)
    sumsq = outer.tile([128, 32], F32)
    spill_v = spill.ap().rearrange("(tt p) (q c4) -> p tt q c4", p=128, c4=512)

    es2 = ExitStack()
    zpool = es2.enter_context(tc.tile_pool(name="zpool", bufs=1))
    qq_sb = zpool.tile([128, N], BF16)
    qkk_sb = zpool.tile([128, N], BF16)
    zT8 = []
    for c in range(8):
        zT8c = zpool.tile([128, 4, 512], FP8, tag=f"zT8_{c}")
        zT8.append(zT8c)

    # ---------------- P0: token-shifted LayerNorm -> zT8 chunks ----------------
    # x loads batched 4 token-tiles per DMA; shifted first-half channels loaded
    # separately with a one-row offset.
    xs_v = x.ap().rearrange("(c p) d -> p c d", p=128)
    with tc.tile_pool(name="p0", bufs=4) as p0, \
         tc.tile_pool(name="p0z", bufs=4) as p0z, \
         tc.tile_pool(name="p0s", bufs=8) as p0s:
        for cch in (0, 1, 2, 3, 4, 5, 6, 7):
            x4 = p0.tile([128, 4, D], F32, tag="x4")
            t0 = cch * 512
            if cch == 0:
                nc.vector.memset(x4[0:1, 0, 0:D // 2], 0.0)
                nc.gpsimd.dma_start(x4[1:128, 0, 0:D // 2], x[0:127, 0:D // 2])
                for j in range(1, 4):
                    nc.gpsimd.dma_start(x4[:, j, 0:D // 2],
                                        x[t0 + j * 128 - 1:t0 + j * 128 + 127, 0:D // 2])
            else:
                nc.gpsimd.dma_start(
                    x4[:, :, 0:D // 2],
                    x.ap()[t0 - 1:t0 + 511, 0:D // 2].rearrange("(j p) d -> p j d", p=128))
            nc.gpsimd.dma_start(x4[:, :, D // 2:D],
                                xs_v[:, 4 * cch:4 * cch + 4, D // 2:D])
            ztc = p0z.tile([128, 4, 512], BF16, tag="ztc")
            for j in range(4):
                st6 = p0s.tile([128, 6], F32, tag="st6")
                nc.vector.bn_stats(st6[:, :], x4[:, j, :])
                mv = p0s.tile([128, 2], F32, tag="mv")
                nc.vector.bn_aggr(mv[:, :], st6[:, :])
                rstd = p0s.tile([128, 1], F32, tag="rstd")
                nc.scalar.activation(rstd[:, :], mv[:, 1:2], AF.Sqrt, bias=eps_sb[:, :], scale=1.0)
                nc.vector.reciprocal(rstd[:, :], rstd[:, :])
                nmu = p0s.tile([128, 1], F32, tag="nmu")
                nc.vector.tensor_scalar(out=nmu[:, :], in0=mv[:, 0:1], scalar1=rstd[:, :],
                                        scalar2=-1.0, op0=OP.mult, op1=OP.mult)
                zt = p0.tile([128, D], BF16, tag="zt")
                nc.scalar.activation(zt[:, :], x4[:, j, :], AF.Identity,
                                     bias=nmu[:, :], scale=rstd[:, :])
                eng = nc.sync if j % 2 == 0 else nc.scalar
                eng.dma_start_transpose(ztc[:, :, j * 128:j * 128 + 128], zt[:, :])
                nc.scalar.activation(zT8[cch][:, :, j * 128:j * 128 + 128],
                                     ztc[:, :, j * 128:j * 128 + 128], AF.Copy)

    # ---------------- P1+P3: qk path interleaved with hidden FFConvM ----------------
    with tc.tile_pool(name="p1", bufs=1) as p1, \
         tc.tile_pool(name="p3", bufs=2) as p3, \
         tc.tile_pool(name="p3d", bufs=3) as p3d, \
         tc.tile_pool(name="p3q", bufs=1) as p3q, \
         tc.tile_pool(name="p3v", bufs=2) as p3v, \
         tc.tile_pool(name="p1s", bufs=3) as p1s, \
         tc.tile_pool(name="p3p", bufs=3, space="PSUM") as p3p, \
         tc.tile_pool(name="p3cp", bufs=3, space="PSUM") as p3cp, \
         tc.tile_pool(name="p1sp", bufs=1, space="PSUM") as p1sp, \
         tc.tile_pool(name="p3lin", bufs=1, space="PSUM") as p3lin:
        state = {"strips4": None}

        def produce_start(hc):
            dg = p3d.tile([128, 7, 2, 128], FP8, tag="dg")
            nc.gpsimd.dma_start(dg[:, :, :, :], diag.ap()[:, hc, :, :, :])
            hb = p3.tile([128, NPADBUF], BF16, tag="hpad")
            nc.vector.memset(hb[:, 0:PAD], 0.0)
            nc.vector.memset(hb[:, PAD + N:], 0.0)
            return dg, hb

        def produce_chunk(hc, hb, c):
            ps = p3p.tile([128, 512], F32, tag="hps")
            for pr in range(2):
                nc.tensor.matmul(ps[:, :], wh_sb[:, pr, :, hc * 128:(hc + 1) * 128],
                                 zT8[c][:, 2 * pr:2 * pr + 2, :],
                                 start=(pr == 0), stop=(pr == 1), perf_mode=DR)
            nc.scalar.activation(hb[:, PAD + c * 512:PAD + (c + 1) * 512], ps[:, :],
                                 AF.Silu, bias=bh_sb[:, hc:hc + 1], scale=1.0)

        def produce_end(hb):
            h8 = p3.tile([128, NPADBUF], FP8, tag="h8pad")
            nc.scalar.activation(h8[:, :], hb[:, :], AF.Copy)
            return h8

        def produce(hc):
            dg, hb = produce_start(hc)
            for c in range(8):
                produce_chunk(hc, hb, c)
            return dg, hb, produce_end(hb)

        def convpost(hc, dg, hb, h8):
            if hc % 4 == 0:
                s4_new = p3q.tile([128, NT, 4, 128], BF16, tag="strips4")
                state["strips4"] = s4_new
            strips4 = state["strips4"]
            acc = p3.tile([128, N], BF16, tag="acc")
            _emit_conv(nc, p3cp, p3v, dg, dwv_sb[:, hc, :], h8, hb, acc)
            nc.sync.dma_start_transpose(strips4[:, :, hc % 4, :], acc[:, :])
            if hc % 4 == 3:
                q = hc // 4
                nc.gpsimd.dma_start(spill_v[:, :, q, :], strips4[:, :, :, :])
                kvp = p3lin.tile([128, 512], F32, tag="kvps")
                for tt in range(NT):
                    nc.tensor.matmul(
                        kvp[:, :], lk_str[:, tt, :],
                        strips4[:, tt, :, :].rearrange("p a c -> p (a c)"),
                        start=(tt == 0), stop=(tt == NT - 1))
                dst = linkv_sb if q < 2 else linku_sb
                nc.scalar.activation(dst[:, (q % 2) * 512:(q % 2) * 512 + 512],
                                     kvp[:, :], AF.Copy)

        # Front: chunk-major over {qk, hc0, hc1} so the PE consumes zT8 chunks
        # the moment P0 produces them (no head-of-line blocking on later chunks).
        dgq = p3d.tile([128, 7, 2, 128], FP8, tag="dg")
        nc.gpsimd.dma_start(dgq[:, :, :, :], diag.ap()[:, 20, :, :, :])
        qkp = p1.tile([128, NPADBUF], BF16, tag="qkpad")
        nc.vector.memset(qkp[:, 0:PAD], 0.0)
        nc.vector.memset(qkp[:, PAD + N:], 0.0)
        q8p = p1.tile([128, NPADBUF], FP8, tag="qk8pad")
        dg0, hb0 = produce_start(0)
        dg1, hb1 = produce_start(1)
        for c in range(8):
            ps = p3p.tile([128, 512], F32, tag="hps")
            for pr in range(2):
                nc.tensor.matmul(ps[:, :], wqk_sb[:, pr, :, :],
                                 zT8[c][:, 2 * pr:2 * pr + 2, :],
                                 start=(pr == 0), stop=(pr == 1), perf_mode=DR)
            nc.scalar.activation(qkp[:, PAD + c * 512:PAD + (c + 1) * 512], ps[:, :],
                                 AF.Silu, bias=bqk_sb[:, :], scale=1.0)
            produce_chunk(0, hb0, c)
            produce_chunk(1, hb1, c)
        nc.scalar.activation(q8p[:, :], qkp[:, :], AF.Copy)
        h80 = produce_end(hb0)
        h81 = produce_end(hb1)
        qkc = p3.tile([128, N], BF16, tag="acc")
        _emit_conv(nc, p3cp, p3v, dgq, dwv_sb[:, 20, :], q8p, qkp, qkc)
        lkk = p3.tile([128, N], BF16, tag="acc")
        for i, dst in ((0, qq_sb), (1, lq_sb), (2, qkk_sb), (3, lkk)):
            nc.vector.tensor_scalar(out=dst[:, :], in0=qkc[:, :], scalar1=gb_sb[:, i:i + 1],
                                    scalar2=gb_sb[:, 4 + i:5 + i], op0=OP.mult, op1=OP.add)
        nc.sync.dma_start_transpose(lk_str[:, :, :], lkk[:, :])
        convpost(0, dg0, hb0, h80)
        for g in range(NG):
            for jh in range(2):
                sp = p1sp.tile([128, G], F32, tag="simps")
                nc.tensor.matmul(sp[:, :],
                                 qkk_sb[:, g * G + jh * 128: g * G + jh * 128 + 128],
                                 qq_sb[:, g * G:(g + 1) * G],
                                 start=True, stop=True)
                rel = p1s.tile([128, G], BF16, tag="rel")
                nc.scalar.activation(rel[:, :], sp[:, :], AF.Relu, scale=RELUSC)
                nc.scalar.activation(attnT8[:, g, jh, :], rel[:, :], AF.Square)

        prev = (1, dg1, hb1, h81)
        for hc in range(2, 16):
            pr = produce(hc)
            convpost(*prev)
            prev = (hc, *pr)
        convpost(*prev)

    es2.close()  # frees zT8 chunks before the P4/P5 pools open

    # ---------------- P4: attention apply + gating (deferred LN) ----------------
    with tc.tile_pool(name="mid", bufs=1) as mid:
        zoT8 = mid.tile([128, 8, N], FP8)
        vo_big = mid.tile([128, NT, 4, 128], BF16)
        with tc.tile_pool(name="p4", bufs=2) as p4, \
             tc.tile_pool(name="p4g", bufs=GRP + 2) as p4g, \
             tc.tile_pool(name="p4s", bufs=3) as p4s, \
             tc.tile_pool(name="p4p", bufs=2, space="PSUM") as p4p:
            govu = []   # (go, vt?, ...) per pending it in current group
            for g in range(NG):
                vt, ut = [], []
                for jh in range(2):
                    vtj = p4.tile([128, E2], BF16, tag=f"vg{jh}")
                    nc.gpsimd.dma_start(vtj[:, :], spill[g * G + jh * 128: g * G + jh * 128 + 128, 0:E2])
                    utj = p4.tile([128, E2], BF16, tag=f"ug{jh}")
                    nc.gpsimd.dma_start(utj[:, :], spill[g * G + jh * 128: g * G + jh * 128 + 128, E2:H])
                    vt.append(vtj)
                    ut.append(utj)
                vt8 = p4.tile([128, 2, 2, 512], FP8, tag="vt8")
                ut8 = p4.tile([128, 2, 2, 512], FP8, tag="ut8")
                for jh in range(2):
                    for e in range(2):
                        nc.vector.tensor_copy(vt8[:, e, jh, :], vt[jh][:, e * 512:(e + 1) * 512])
                        nc.scalar.activation(ut8[:, e, jh, :], ut[jh][:, e * 512:(e + 1) * 512],
                                             AF.Copy)
                for it in range(2):
                    idx = g * 2 + it
                    islice = slice(g * G + it * 128, g * G + it * 128 + 128)
                    avp = p4p.tile([128, E2], F32, tag="avps")
                    aup = p4p.tile([128, E2], F32, tag="aups")
                    for dst, m8, lin in ((avp, vt8, linkv_sb), (aup, ut8, linku_sb)):
                        for e in range(2):
                            nc.tensor.matmul(dst[:, e * 512:(e + 1) * 512],
                                             attnT8[:, g, :, it * 128:it * 128 + 128],
                                             m8[:, e, :, :],
                                             start=True, stop=False, perf_mode=DR)
                            nc.tensor.matmul(dst[:, e * 512:(e + 1) * 512],
                                             lq_sb[:, islice], lin[:, e * 512:(e + 1) * 512],
                                             start=False, stop=True)
                    t1 = p4s.tile([128, E2], BF16, tag="t1")
                    nc.vector.scalar_tensor_tensor(out=t1[:, :], in0=avp[:, :],
                                                   scalar=1.0 / ASCALE, in1=ut[it][:, :],
                                                   op0=OP.mult, op1=OP.mult)
                    sg = p4s.tile([128, E2], BF16, tag="sg")
                    nc.scalar.activation(sg[:, :], t1[:, :], AF.Sigmoid)
                    t2 = p4s.tile([128, E2], BF16, tag="t2")
                    nc.vector.scalar_tensor_tensor(out=t2[:, :], in0=aup[:, :],
                                                   scalar=1.0 / ASCALE, in1=vt[it][:, :],
                                                   op0=OP.mult, op1=OP.mult)
                    go = p4g.tile([128, E2], BF16, tag="go")
                    nc.vector.scalar_tensor_tensor(out=go[:, :], in0=t2[:, :], scalar=1.0,
                                                   in1=sg[:, :], op0=OP.mult, op1=OP.mult,
                                                   accum_out=sums[:, idx:idx + 1])
                    jnk = p4s.tile([128, E2], BF16, tag="jnk")
                    nc.scalar.activation(jnk[:, :], go[:, :], AF.Square,
                                         accum_out=sumsq[:, idx:idx + 1])
                    govu.append(go)
                    if len(govu) == GRP:
                        _p4_norm(nc, tc, p4s, govu, sums, sumsq, eps_sb, zoT8,
                                 idx - GRP + 1)
                        govu = []

        # ---------------- P5: output FFConvM ----------------
        with tc.tile_pool(name="p5", bufs=2) as p5, \
             tc.tile_pool(name="p5d", bufs=2) as p5d, \
             tc.tile_pool(name="p5v", bufs=2) as p5v, \
             tc.tile_pool(name="p5p", bufs=2, space="PSUM") as p5p, \
             tc.tile_pool(name="p5cp", bufs=3, space="PSUM") as p5cp:
            def produce5(oc):
                dg = p5d.tile([128, 7, 2, 128], FP8, tag="dg5")
                nc.gpsimd.dma_start(dg[:, :, :, :], diag.ap()[:, 16 + oc, :, :, :])
                hb = p5.tile([128, NPADBUF], BF16, tag="hpad5")
                nc.vector.memset(hb[:, 0:PAD], 0.0)
                nc.vector.memset(hb[:, PAD + N:], 0.0)
                for c in range(8):
                    ps = p5p.tile([128, 512], F32, tag="ops")
                    for pr in range(4):
                        nc.tensor.matmul(ps[:, :], wo_sb[:, pr, :, oc * 128:(oc + 1) * 128],
                                         zoT8[:, 2 * pr:2 * pr + 2, c * 512:(c + 1) * 512],
                                         start=(pr == 0), stop=(pr == 3), perf_mode=DR)
                    nc.scalar.activation(hb[:, PAD + c * 512:PAD + (c + 1) * 512], ps[:, :],
                                         AF.Silu, bias=bo_sb[:, oc:oc + 1], scale=1.0)
                h8 = p5.tile([128, NPADBUF], FP8, tag="h85")
                nc.scalar.activation(h8[:, :], hb[:, :], AF.Copy)
                return dg, hb, h8

            def convpost5(oc, dg, hb, h8):
                acc = p5.tile([128, N], BF16, tag="acc5")
                _emit_conv(nc, p5cp, p5v, dg, dwv_sb[:, 16 + oc, :], h8, hb, acc)
                nc.sync.dma_start_transpose(vo_big[:, :, oc, :], acc[:, :])

            prev = None
            for oc in range(4):
                pr = produce5(oc)
                if prev is not None:
                    convpost5(*prev)
                prev = (oc, *pr)
            convpost5(*prev)

        # ---------------- P6: residual (4 token-tiles per iteration) ----------------
        out_v = out.ap().rearrange("(c p) d -> p c d", p=128)
        with tc.tile_pool(name="p6", bufs=3) as p6:
            for cch in range(8):
                xt = p6.tile([128, 4, D], F32, tag="xt6")
                nc.gpsimd.dma_start(xt[:, :, :], xs_v[:, 4 * cch:4 * cch + 4, :])
                of = p6.tile([128, 4, D], F32, tag="of")
                nc.vector.tensor_tensor(
                    out=of[:, :, :].rearrange("p a c -> p (a c)"),
                    in0=xt[:, :, :].rearrange("p a c -> p (a c)"),
                    in1=vo_big[:, 4 * cch:4 * cch + 4, :, :].rearrange("p a b c -> p (a b c)"),
                    op=OP.add)
                nc.gpsimd.dma_start(out_v[:, 4 * cch:4 * cch + 4, :], of[:, :, :])
    es.close()


def _p4_norm(nc, tc, pool, gos, sums, sumsq, eps_sb, zoT8, idx0):
    """Deferred LayerNorm for GRP gating tiles: batched stats then per-tile
    normalize + transpose + fp8 convert."""
    n = len(gos)
    sl = slice(idx0, idx0 + n)
    mean = pool.tile([128, n], F32, tag="mean")
    nc.vector.tensor_scalar(out=mean[:, :], in0=sums[:, sl], scalar1=1.0 / E2,
                            scalar2=None, op0=OP.mult)
    msq = pool.tile([128, n], F32, tag="msq")
    nc.vector.tensor_tensor(out=msq[:, :], in0=mean[:, :], in1=mean[:, :], op=OP.mult)
    var = pool.tile([128, n], F32, tag="var")
    nc.vector.scalar_tensor_tensor(out=var[:, :], in0=sumsq[:, sl], scalar=1.0 / E2,
                                   in1=msq[:, :], op0=OP.mult, op1=OP.subtract)
    rstd = pool.tile([128, n], F32, tag="rstdn")
    nc.scalar.activation(rstd[:, :], var[:, :], AF.Sqrt, bias=eps_sb[:, :], scale=1.0)
    nc.vector.reciprocal(rstd[:, :], rstd[:, :])
    nmu = pool.tile([128, n], F32, tag="nmun")
    nc.vector.tensor_tensor(out=nmu[:, :], in0=mean[:, :], in1=rstd[:, :], op=OP.mult)
    nc.vector.tensor_scalar(out=nmu[:, :], in0=nmu[:, :], scalar1=-1.0,
                            scalar2=None, op0=OP.mult)
    for j, go in enumerate(gos):
        tti = idx0 + j
        zo = pool.tile([128, E2], BF16, tag="zon")
        nc.vector.tensor_scalar(out=zo[:, :], in0=go[:, :], scalar1=rstd[:, j:j + 1],
                                scalar2=nmu[:, j:j + 1], op0=OP.mult, op1=OP.add)
        zot = pool.tile([128, 8, 128], BF16, tag="zot")
        nc.sync.dma_start_transpose(zot[:, :, :], zo[:, :])
        if j % 2 == 0:
            nc.scalar.activation(zoT8[:, :, tti * 128:(tti + 1) * 128], zot[:, :, :], AF.Copy)
        else:
            nc.vector.tensor_copy(zoT8[:, :, tti * 128:(tti + 1) * 128], zot[:, :, :])


def _build_nc():
    nc = bacc.Bacc("TRN2", target_bir_lowering=False, debug=False)
    x = nc.dram_tensor("x", [N, D], F32, kind="ExternalInput")
    wh8 = nc.dram_tensor("wh8", [128, 2, 2, H], FP8, kind="ExternalInput")
    wqk8 = nc.dram_tensor("wqk8", [128, 2, 2, QK], FP8, kind="ExternalInput")
    wo8 = nc.dram_tensor("wo8", [128, 4, 2, D], FP8, kind="ExternalInput")
    bh = nc.dram_tensor("bh", [128, 16], F32, kind="ExternalInput")
    bqk = nc.dram_tensor("bqk", [128, 1], F32, kind="ExternalInput")
    bo = nc.dram_tensor("bo", [128, 4], F32, kind="ExternalInput")
    gb = nc.dram_tensor("gb", [128, 8], F32, kind="ExternalInput")
    diag = nc.dram_tensor("diag", [128, NCH, 7, 2, 128], FP8, kind="ExternalInput")
    dwv = nc.dram_tensor("dwv", [128, NCH, 4], F32, kind="ExternalInput")
    out = nc.dram_tensor("out", [N, D], F32, kind="ExternalOutput")
    spill = nc.dram_tensor("spill", [N, H], BF16)
    with tile.TileContext(nc) as tc:
        _emit(nc, tc, x, wh8, wqk8, wo8, bh, bqk, bo, gb, diag, dwv, out, spill)
    nc.compile()
    return nc


def prep_inputs(inputs):
    f32 = np.float32
    fp8 = ml_dtypes.float8_e4m3
    W_h = np.asarray(inputs["W_h"], f32)
    W_qk = np.asarray(inputs["W_qk"], f32)
    W_o = np.asarray(inputs["W_o"], f32)
    whp = np.asarray(inputs["ln_h_g"], f32)[:, None] * W_h
    bhp = np.asarray(inputs["ln_h_b"], f32) @ W_h + np.asarray(inputs["b_h"], f32)
    wqkp = np.asarray(inputs["ln_qk_g"], f32)[:, None] * W_qk
    bqkp = np.asarray(inputs["ln_qk_b"], f32) @ W_qk + np.asarray(inputs["b_qk"], f32)
    wop = np.asarray(inputs["ln_o_g"], f32)[:, None] * W_o
    bop = np.asarray(inputs["ln_o_b"], f32) @ W_o + np.asarray(inputs["b_o"], f32)
    gamma = np.asarray(inputs["gamma"], f32).copy()
    beta = np.asarray(inputs["beta"], f32).copy()
    gamma[0] /= G
    beta[0] /= G
    gamma[1] *= ASCALE
    beta[1] *= ASCALE
    gamma[3] /= N
    beta[3] /= N

    def lhsT8(w, kt):
        # [din, dout] -> [128, kt/2 pairs, 2, dout] fp8
        t = w.reshape(kt, 128, -1).transpose(1, 0, 2)  # [128, kt, dout]
        return np.ascontiguousarray(
            t.reshape(128, kt // 2, 2, t.shape[-1])).astype(fp8)

    def chan(v, ntiles):
        return np.ascontiguousarray(v.reshape(ntiles, 128).T).astype(f32)

    # diagonal conv stationaries: [128, NCH, 9, 2, 128] fp8
    dw_h = np.asarray(inputs["dw_h"], f32)
    dw_o = np.asarray(inputs["dw_o"], f32)
    dw_qk = np.asarray(inputs["dw_qk"], f32)
    diag = np.zeros((128, NCH, 7, 2, 128), f32)
    dwv = np.zeros((128, NCH, 4), f32)
    rng128 = np.arange(128)
    for ct in range(NCH):
        if ct < 16:
            wsrc = dw_h[:, ct * 128:(ct + 1) * 128]
        elif ct < 20:
            wsrc = dw_o[:, (ct - 16) * 128:(ct - 15) * 128]
        else:
            wsrc = dw_qk
        for pr, (k0, k1) in enumerate(PAIRS):
            diag[rng128, ct, pr, 0, rng128] = wsrc[k0]
            diag[rng128, ct, pr, 1, rng128] = wsrc[k1]
        diag[rng128, ct, 6, 0, rng128] = wsrc[16]
        for j, k in enumerate(DVETAPS):
            dwv[:, ct, j] = wsrc[k]
    return {
        "wh8": lhsT8(whp, 4), "wqk8": lhsT8(wqkp, 4), "wo8": lhsT8(wop, 8),
        "bh": chan(bhp, 16), "bqk": chan(bqkp, 1), "bo": chan(bop, 4),
        "gb": np.concatenate([gamma.T, beta.T], axis=1).astype(f32),
        "diag": diag.astype(fp8), "dwv": dwv,
    }


_NC = None


def get_nc():
    global _NC
    if _NC is None:
        _NC = _build_nc()
    return _NC


def make_in_maps(inputs):
    x = np.asarray(inputs["x"], np.float32)
    B = x.shape[0]
    prep = prep_inputs(inputs)
    return [{"x": np.ascontiguousarray(x[b]), **prep} for b in range(B)]


def kernel(**inputs):
    nc = get_nc()
    in_maps = make_in_maps(inputs)
    res = bass_utils.run_bass_kernel_spmd(nc, in_maps, core_ids=list(range(8)))
    out = np.stack([res.results[b]["out"] for b in range(8)], axis=0)
    return out.astype(np.float32)
